# revision 1
# baseline (speedup 1.0000x reference)
"""DeepGCN (GENConv x3, softmax aggregation) on 8 Trainium2 NeuronCores.

Strategy (edge-parallel, dst-sharded):
  - Nodes are dst-sharded across 8 cores; every edge lives on the core owning
    its dst, so segment-softmax stats need no cross-core combine.
  - Softmax aggregation without segment_max (mathematically identical since
    softmax is shift-invariant and msg >= 0):
        msg = relu(y), w = exp(t*msg) = max(exp(t*y), 1), v = msg*w = relu(y)*exp(t*y)
        agg = seg_sum(v) / seg_sum(w)
    seg_sum via TensorE: per 128-edge block, matmul lhsT=[w-1 | v] (128x128
    bf16) against one-hot dst-slot indicators S (128x32 bf16), accumulating
    into a PSUM group window; the "-1" is fixed by accumulating +deg via a
    K=1 matmul of the per-slot in-degree.
  - Node-level tensors live in per-core "slot space" (windows of 32 slots,
    <=768 edges and one graph per window) so the program is identical on all
    cores (SPMD); all per-core variation is input data.
  - x[src] fetched with dma_gather (256B rows) from a replicated slot-space
    table; the int16 index limit is handled with two overlapping table views
    (lo/hi) plus per-window exact lo/hi section balancing using edges whose
    src slot both views reach.
  - GENConv MLP + LayerNorms run channel-major: partition reductions via
    ones-matmuls, per-node affine replicated via K=1 matmuls.
  - Readout: per-window masked max, per-graph max via additive -inf masks,
    AllReduce(max), sigmoid(pooled @ ro_w + ro_b).
"""

import sys

sys.path.insert(0, "/opt/trn_rl_repo")

import numpy as np
import ml_dtypes

import concourse.bass as bass
import concourse.bacc as bacc
import concourse.mybir as mybir
import concourse.tile as tile
from concourse.bass_utils import run_bass_kernel_spmd

F32 = mybir.dt.float32
BF16 = mybir.dt.bfloat16
I16 = mybir.dt.int16
AF = mybir.ActivationFunctionType
OP = mybir.AluOpType
AX = mybir.AxisListType

NC = 8
H = 64
F_NODE = 128
F_EDGE = 32
HID = 128
L = 3
NEG_BIG = -1.0e30
N_GRAPHS = 64

W_SLOTS = 32          # slots (nodes) per window
WIN_BLOCKS = 6        # 128-edge blocks per window
T_LO = 3              # lo-section blocks per window
T_HI = WIN_BLOCKS - T_LO
BLK = 128
WIN_EDGES = WIN_BLOCKS * BLK          # 768
SEC_LO = T_LO * BLK                   # 384
SEC_HI = T_HI * BLK
GRP_WIN = 8           # windows per PSUM group
GRP_SLOTS = GRP_WIN * W_SLOTS         # 256
GRP_EDGES = GRP_WIN * WIN_EDGES       # 6144
GATHER_LIMIT = 32768  # int16 gather index reach
STOP_AFTER = None  # debug: 'ea','h0','table0','edges0','layer0','layer1','layer2'


class Plan:
    pass


class _StopBuild(Exception):
    pass


import contextlib


@contextlib.contextmanager
def _nullcm():
    yield


# ----------------------------------------------------------------------------
# host-side planning
# ----------------------------------------------------------------------------

def build_plan(edge_index, batch, n_nodes):
    src = edge_index[0].astype(np.int64)
    dst = edge_index[1].astype(np.int64)
    npc = n_nodes // NC

    deg = np.bincount(dst, minlength=n_nodes)

    def pack(core):
        wins = []
        cur = []
        cur_e = 0
        for n in range(core * npc, (core + 1) * npc):
            d = int(deg[n])
            if cur and (
                cur_e + d > WIN_EDGES
                or len(cur) >= W_SLOTS
                or batch[n] != batch[cur[0]]
            ):
                wins.append(cur)
                cur, cur_e = [], 0
            cur.append(n)
            cur_e += d
        if cur:
            wins.append(cur)
        return wins

    core_wins = [pack(c) for c in range(NC)]

    w_prog = max(len(w) for w in core_wins)
    w_prog = ((w_prog + GRP_WIN - 1) // GRP_WIN) * GRP_WIN
    s_core = w_prog * W_SLOTS
    s_global = s_core * NC
    lo_rows = min(s_global, GATHER_LIMIT)
    hi_base = max(0, s_global - GATHER_LIMIT)
    hi_rows = s_global - hi_base
    assert hi_rows <= GATHER_LIMIT and lo_rows <= GATHER_LIMIT, (
        f"slot space too large: {s_global}"
    )

    slot_of = np.full(n_nodes, -1, np.int64)
    win_of_node = np.full(n_nodes, -1, np.int64)
    graph_of_win = np.full((NC, w_prog), -1, np.int64)
    for c in range(NC):
        for w, nodes in enumerate(core_wins[c]):
            base = c * s_core + w * W_SLOTS
            for j, n in enumerate(nodes):
                slot_of[n] = base + j
                win_of_node[n] = w
            graph_of_win[c, w] = batch[nodes[0]]

    src_slot = slot_of[src]
    assert (src_slot >= 0).all()
    edge_win = win_of_node[dst]
    core_of = dst // npc

    e_prog = w_prog * WIN_EDGES
    n_groups = w_prog // GRP_WIN

    plans = []
    for c in range(NC):
        e_ids = np.nonzero(core_of == c)[0]
        win_edges = [[] for _ in range(w_prog)]
        for e in e_ids:
            win_edges[edge_win[e]].append(e)

        perm = np.full(e_prog, -1, np.int64)
        scol = np.full(e_prog, -1, np.int64)
        gidx = np.zeros(e_prog, np.int64)

        for w in range(w_prog):
            ew = np.array(win_edges[w], np.int64)
            base = w * WIN_EDGES
            if not ew.size:
                continue
            ss = src_slot[ew]
            f_lo = ss < hi_base
            f_hi = ss >= lo_rows
            flex = ~(f_lo | f_hi)
            n_t, n_fl, n_fh = ew.size, int(f_lo.sum()), int(f_hi.sum())
            assert n_t <= WIN_EDGES
            assert n_fl <= SEC_LO, f"lo overflow c{c} w{w}: {n_fl}"
            assert n_fh <= SEC_HI, f"hi overflow c{c} w{w}: {n_fh}"
            n_lo = min(SEC_LO, n_t - n_fh)
            lo_ids = np.concatenate([ew[f_lo], ew[flex][: n_lo - n_fl]])
            hi_ids = np.concatenate([ew[flex][n_lo - n_fl :], ew[f_hi]])
            assert lo_ids.size == n_lo and hi_ids.size == n_t - n_lo <= SEC_HI
            lo_ids = lo_ids[np.argsort(src_slot[lo_ids], kind="stable")]
            hi_ids = hi_ids[np.argsort(src_slot[hi_ids], kind="stable")]
            wbase = c * s_core + w * W_SLOTS
            perm[base : base + n_lo] = lo_ids
            scol[base : base + n_lo] = slot_of[dst[lo_ids]] - wbase
            gidx[base : base + n_lo] = src_slot[lo_ids]
            hb = base + SEC_LO
            perm[hb : hb + hi_ids.size] = hi_ids
            scol[hb : hb + hi_ids.size] = slot_of[dst[hi_ids]] - wbase
            gidx[hb : hb + hi_ids.size] = src_slot[hi_ids] - hi_base

        # stream order: per group, the 8 windows' lo sections, then hi sections
        pos = np.arange(e_prog)
        w_all = pos // WIN_EDGES
        off = pos % WIN_EDGES
        g = w_all // GRP_WIN
        wl = w_all % GRP_WIN
        is_lo = off < SEC_LO
        new_pos = np.where(
            is_lo,
            g * GRP_EDGES + wl * SEC_LO + off,
            g * GRP_EDGES + GRP_WIN * SEC_LO + wl * SEC_HI + (off - SEC_LO),
        )
        p = Plan()
        p.perm = np.full(e_prog, -1, np.int64)
        p.scol = np.full(e_prog, -1, np.int64)
        p.gidx = np.zeros(e_prog, np.int64)
        p.perm[new_pos] = perm
        p.scol[new_pos] = scol
        p.gidx[new_pos] = gidx
        plans.append(p)

    g = Plan()
    g.w_prog, g.s_core, g.s_global = w_prog, s_core, s_global
    g.lo_rows, g.hi_base, g.hi_rows = lo_rows, hi_base, hi_rows
    g.e_prog, g.n_groups, g.npc = e_prog, n_groups, npc
    g.slot_of, g.deg = slot_of, deg
    g.core_wins, g.graph_of_win = core_wins, graph_of_win
    g.plans = plans
    g.n_nodes = n_nodes
    return g


def wrap_idx(idx_flat):
    n = idx_flat.shape[0]
    arr = np.zeros((128, n // 16), np.int16)
    arr[np.arange(n) % 16, np.arange(n) // 16] = idx_flat.astype(np.int16)
    for r in range(1, 8):
        arr[16 * r : 16 * (r + 1)] = arr[0:16]
    return arr


def build_core_inputs(gp, inputs):
    x = np.asarray(inputs["x"], np.float32)
    edge_attr = np.asarray(inputs["edge_attr"], np.float32)

    shared = {
        "ne_wT": np.ascontiguousarray(np.asarray(inputs["ne_w"], np.float32).T),
        "ne_b": np.asarray(inputs["ne_b"], np.float32).reshape(H, 1),
        "eaw1b": np.concatenate(
            [
                np.asarray(inputs["ee_w1"], np.float32).T,
                np.asarray(inputs["ee_b1"], np.float32).reshape(1, -1),
            ]
        ),
        "eaw2b": np.concatenate(
            [
                np.asarray(inputs["ee_w2"], np.float32).T,
                np.asarray(inputs["ee_b2"], np.float32).reshape(1, -1),
            ]
        ),
        "eye": np.eye(128, dtype=np.float32),
        "ro_w": np.ascontiguousarray(
            np.asarray(inputs["ro_w"], np.float32).reshape(1, H).T
        ),
        "ro_b": np.full(
            (N_GRAPHS, 1), float(np.asarray(inputs["ro_b"]).reshape(-1)[0]), np.float32
        ),
    }
    for l in range(L):
        shared[f"w1b{l}"] = np.concatenate(
            [
                np.asarray(inputs["conv_w1"], np.float32)[l].T,
                np.asarray(inputs["conv_b1"], np.float32)[l].reshape(1, -1),
            ]
        )
        shared[f"w2T{l}"] = np.ascontiguousarray(
            np.asarray(inputs["conv_w2"], np.float32)[l].T
        )
        shared[f"b2_{l}"] = np.asarray(inputs["conv_b2"], np.float32)[l].reshape(H, 1)
        shared[f"g_{l}"] = np.asarray(inputs["conv_g"], np.float32)[l].reshape(HID, 1)
        shared[f"bn_{l}"] = np.asarray(inputs["conv_bn"], np.float32)[l].reshape(HID, 1)
        shared[f"lng{l}"] = np.asarray(inputs["ln_g"], np.float32)[l].reshape(H, 1)
        shared[f"lnb{l}"] = np.asarray(inputs["ln_b"], np.float32)[l].reshape(H, 1)

    core_maps = []
    n_lo = GRP_WIN * SEC_LO
    for c in range(NC):
        p = gp.plans[c]
        valid = p.perm >= 0
        perm_safe = np.where(valid, p.perm, 0)

        idx_arr = np.zeros((128, gp.e_prog // 16), np.int16)
        for grp in range(gp.n_groups):
            b0 = grp * GRP_EDGES
            idx_arr[:, b0 // 16 : (b0 + n_lo) // 16] = wrap_idx(
                p.gidx[b0 : b0 + n_lo]
            )
            idx_arr[:, (b0 + n_lo) // 16 : (b0 + GRP_EDGES) // 16] = wrap_idx(
                p.gidx[b0 + n_lo : b0 + GRP_EDGES]
            )

        S = np.zeros((gp.e_prog, W_SLOTS), np.float32)
        pos = np.arange(gp.e_prog)
        S[pos[valid], p.scol[valid]] = 1.0
        S = S.reshape(gp.e_prog // BLK, BLK, W_SLOTS).astype(ml_dtypes.bfloat16)

        ea = edge_attr[perm_safe] * valid[:, None]
        eaT33 = np.concatenate(
            [ea.T, np.ones((1, gp.e_prog), np.float32)]
        ).astype(np.float32)

        node_of_slot = np.full(gp.s_core, -1, np.int64)
        for w, nodes in enumerate(gp.core_wins[c]):
            for j, n in enumerate(nodes):
                node_of_slot[w * W_SLOTS + j] = n
        sv = node_of_slot >= 0
        ns = np.where(sv, node_of_slot, 0)
        degp = np.where(sv, np.maximum(gp.deg[ns], 1), 1).astype(np.float32)

        M = np.full((N_GRAPHS, gp.w_prog), NEG_BIG, np.float32)
        for w in range(gp.w_prog):
            gw = gp.graph_of_win[c, w]
            if gw >= 0:
                M[gw, w] = 0.0

        m = dict(shared)
        m.update(
            {
                "idx": idx_arr,
                "S": S,
                "eaT33": eaT33,
                "degb": degp[None, :].astype(ml_dtypes.bfloat16),
                "xT": np.ascontiguousarray(x[ns].T),
                "smask": np.repeat(
                    np.where(sv, 0.0, NEG_BIG).astype(np.float32)[None, :], H, 0
                ),
                "gmask": np.broadcast_to(
                    M[None], (H, N_GRAPHS, gp.w_prog)
                ).copy(),
            }
        )
        core_maps.append(m)
    return core_maps


# ----------------------------------------------------------------------------
# numpy emulation of the device algorithm (validation aid)
# ----------------------------------------------------------------------------

def numpy_forward(gp, core_maps, inputs):
    conv_t = np.asarray(inputs["conv_t"], np.float32)
    bf = lambda a: a.astype(ml_dtypes.bfloat16).astype(np.float32)

    def ln_cm(h, g, b):
        C = h.shape[0]
        mu = h.mean(0, keepdims=True)
        var = (h * h).mean(0, keepdims=True) - mu * mu
        return (h - mu) / np.sqrt(var + 1e-5) * g + b

    z, h, eas = [], [], []
    n_lo = GRP_WIN * SEC_LO
    for c in range(NC):
        m = core_maps[c]
        z.append(m["ne_wT"].T @ m["xT"] + m["ne_b"])
        h.append(np.zeros_like(z[-1]))
        r1 = np.maximum(m["eaw1b"].T @ m["eaT33"], 0)
        r1 = np.concatenate([r1, np.ones((1, gp.e_prog), np.float32)])
        eas.append((m["eaw2b"].T @ r1).T.copy())

    for l in range(L):
        t = float(conv_t[l])
        table = np.concatenate([zz.T for zz in z], axis=0)
        for c in range(NC):
            m = core_maps[c]
            p = gp.plans[c]
            src_rows = np.zeros(gp.e_prog, np.int64)
            for grp in range(gp.n_groups):
                b0 = grp * GRP_EDGES
                src_rows[b0 : b0 + n_lo] = p.gidx[b0 : b0 + n_lo]
                src_rows[b0 + n_lo : b0 + GRP_EDGES] = (
                    p.gidx[b0 + n_lo : b0 + GRP_EDGES] + gp.hi_base
                )
            y = table[src_rows] + eas[c]
            e = np.exp(t * y)
            w1 = bf(np.maximum(e - 1, 0))
            v = bf(np.maximum(y, 0) * e)
            Sb = m["S"].astype(np.float32)
            wv = bf(np.concatenate([w1, v], 1)).reshape(gp.e_prog // BLK, BLK, 2 * H)
            outb = np.einsum("bek,bew->bkw", wv, Sb)
            P = np.zeros((H, gp.s_core), np.float32)
            Q = np.zeros((H, gp.s_core), np.float32)
            bpg = GRP_EDGES // BLK
            for b in range(gp.e_prog // BLK):
                grp, ib = b // bpg, b % bpg
                wl = ib // T_LO if ib < GRP_WIN * T_LO else (ib - GRP_WIN * T_LO) // T_HI
                s0 = grp * GRP_SLOTS + wl * W_SLOTS
                P[:, s0 : s0 + W_SLOTS] += outb[b, 0:H]
                Q[:, s0 : s0 + W_SLOTS] += outb[b, H:]
            P = P + m["degb"].astype(np.float32)
            out_n = Q / P + z[c]
            h1 = m[f"w1b{l}"].T @ np.concatenate(
                [out_n, np.ones((1, gp.s_core), np.float32)]
            )
            z1 = np.maximum(ln_cm(h1, m[f"g_{l}"], m[f"bn_{l}"]), 0)
            h2 = m[f"w2T{l}"].T @ z1 + m[f"b2_{l}"]
            h[c] = h2 if l == 0 else h[c] + h2
            if l < L - 1:
                z[c] = np.maximum(ln_cm(h[c], m[f"lng{l+1}"], m[f"lnb{l+1}"]), 0)

    pooled = np.full((H, N_GRAPHS), NEG_BIG, np.float32)
    for c in range(NC):
        m = core_maps[c]
        q = np.maximum(ln_cm(h[c], m["lng0"], m["lnb0"]), 0) + m["smask"]
        winmax = q.reshape(H, gp.w_prog, W_SLOTS).max(2)
        for gph in range(N_GRAPHS):
            pooled[:, gph] = np.maximum(
                pooled[:, gph], (winmax + m["gmask"][:, gph, :]).max(1)
            )
    r = pooled.T @ core_maps[0]["ro_w"] + core_maps[0]["ro_b"]
    return 1.0 / (1.0 + np.exp(-r))


# ----------------------------------------------------------------------------
# bass program
# ----------------------------------------------------------------------------

def build_nc(gp, conv_t):
    nc = bacc.Bacc("TRN2", debug=False, num_devices=NC, target_bir_lowering=False)

    e_prog, s_core, s_global = gp.e_prog, gp.s_core, gp.s_global
    n_groups, w_prog = gp.n_groups, gp.w_prog
    nblk = e_prog // BLK
    N_LO, N_HI = GRP_WIN * SEC_LO, GRP_WIN * SEC_HI
    BLK_LO, BLK_HI = GRP_WIN * T_LO, GRP_WIN * T_HI
    KCH = GRP_SLOTS // 128  # 128-col chunks per group (= 2)

    din = {}

    def inp(name, shape, dt=F32):
        din[name] = nc.dram_tensor(name, list(shape), dt, kind="ExternalInput")

    inp("idx", [128, e_prog // 16], I16)
    inp("S", [nblk, BLK, W_SLOTS], BF16)
    inp("eaT33", [F_EDGE + 1, e_prog])
    inp("degb", [1, s_core], BF16)
    inp("xT", [F_NODE, s_core])
    inp("smask", [H, s_core])
    inp("gmask", [H, N_GRAPHS, w_prog])
    inp("ne_wT", [F_NODE, H])
    inp("ne_b", [H, 1])
    inp("eaw1b", [F_EDGE + 1, F_EDGE])
    inp("eaw2b", [F_EDGE + 1, H])
    inp("eye", [128, 128])
    inp("ro_w", [H, 1])
    inp("ro_b", [N_GRAPHS, 1])
    for l in range(L):
        inp(f"w1b{l}", [H + 1, HID])
        inp(f"w2T{l}", [HID, H])
        inp(f"b2_{l}", [H, 1])
        inp(f"g_{l}", [HID, 1])
        inp(f"bn_{l}", [HID, 1])
        inp(f"lng{l}", [H, 1])
        inp(f"lnb{l}", [H, 1])

    out_d = nc.dram_tensor("out", [N_GRAPHS, 1], F32, kind="ExternalOutput")

    ea_dram = nc.dram_tensor("ea_edge", [e_prog, H], F32, kind="Internal")
    ea_r = ea_dram.ap().rearrange("(b p) h -> p b h", p=128)
    cc_in = [
        nc.dram_tensor(f"cc_in{l}", [s_core, H], F32, kind="Internal")
        for l in range(L)
    ]
    tables = [
        nc.dram_tensor(
            f"table{l}", [s_global, H], F32, kind="Internal", addr_space="Shared"
        )
        for l in range(L)
    ]
    pool_in = nc.dram_tensor("pool_in", [H, N_GRAPHS], F32, kind="Internal")
    pool_out = nc.dram_tensor(
        "pool_out", [H, N_GRAPHS], F32, kind="Internal", addr_space="Shared"
    )
    rg = [list(range(NC))]

    with tile.TileContext(nc) as tc:
        with tc.tile_pool(name="res", bufs=1) as res:
            idx_t = res.tile([128, e_prog // 16], I16)
            nc.sync.dma_start(idx_t[:], din["idx"].ap())
            z_res = res.tile([H, s_core], F32)
            h_res = res.tile([H, s_core], F32)
            winmax = res.tile([H, w_prog], F32)

            wt = {}
            for name in [
                "ne_wT", "ne_b", "eaw1b", "eaw2b", "eye", "ro_w", "ro_b",
            ] + [
                f"{pre}{l}"
                for l in range(L)
                for pre in ["w1b", "w2T", "b2_", "g_", "bn_", "lng", "lnb"]
            ]:
                wt[name] = res.tile(list(din[name].shape), F32, name=f"wt_{name}")
                nc.sync.dma_start(wt[name][:], din[name].ap())

            ones1x64b = res.tile([1, 2 * H], BF16)
            nc.vector.memset(ones1x64b[:, 0:H], 1.0)
            nc.vector.memset(ones1x64b[:, H:], 0.0)
            ones128 = res.tile([128, 1], F32)
            nc.vector.memset(ones128[:], 1.0)
            ones1x128 = res.tile([1, 128], F32)
            nc.vector.memset(ones1x128[:], 1.0)
            neg1 = res.tile([128, 1], F32)
            nc.vector.memset(neg1[:], -1.0)
            tconst = []
            for l in range(L):
                ct = res.tile([128, 1], F32, name=f"tconst{l}")
                nc.vector.memset(ct[:], float(conv_t[l]))
                tconst.append(ct)

            # ============ phase 1: edge-attr MLP -> ea_dram ============
            CH = 512
            with (
                tc.tile_pool(name="eap", bufs=3) as eap,
                tc.tile_pool(name="eaps", bufs=2, space="PSUM") as eaps,
            ):
                for ch in range(e_prog // CH):
                    sl = slice(ch * CH, (ch + 1) * CH)
                    eat = eap.tile([F_EDGE + 1, CH], F32, tag="eat")
                    nc.sync.dma_start(eat[:], din["eaT33"].ap()[:, sl])
                    ps1 = eaps.tile([F_EDGE, CH], F32, tag="ps1")
                    nc.tensor.matmul(ps1[:], wt["eaw1b"][:], eat[:], start=True, stop=True)
                    r1 = eap.tile([F_EDGE + 1, CH], F32, tag="r1")
                    nc.scalar.activation(r1[0:F_EDGE, :], ps1[:], AF.Relu)
                    nc.vector.memset(r1[F_EDGE : F_EDGE + 1, :], 1.0)
                    ps2 = eaps.tile([128, CH // 128, H], F32, tag="ps2", padded_shape=[128, CH // 128, 128])
                    for k in range(CH // 128):
                        nc.tensor.matmul(
                            ps2[:, k, :],
                            r1[:, k * 128 : (k + 1) * 128],
                            wt["eaw2b"][:],
                            start=(k == 0),
                            stop=(k == CH // 128 - 1),
                        )
                    eo = eap.tile([128, CH // 128, H], F32, tag="eo")
                    nc.vector.tensor_copy(eo[:], ps2[:])
                    nc.sync.dma_start(
                        ea_r[:, ch * (CH // 128) : (ch + 1) * (CH // 128), :], eo[:]
                    )

            stop_now = [False]

            def dummy_out():
                dz = res.tile([N_GRAPHS, 1], F32, name="dummy_out_t")
                nc.vector.memset(dz[:], 0.5)
                nc.sync.dma_start(out_d.ap(), dz[:])

            if STOP_AFTER == "ea":
                stop_now[0] = True
            # ============ phase 2: h0 projection -> z_res ============
            with (
                tc.tile_pool(name="h0sb", bufs=1) as h0sb,
                tc.tile_pool(name="h0ps", bufs=2, space="PSUM") as h0ps,
            ):
                xres = h0sb.tile([F_NODE, s_core], F32, tag="xres")
                nc.sync.dma_start(xres[:], din["xT"].ap())
                for g in range(n_groups):
                    sl = slice(g * GRP_SLOTS, (g + 1) * GRP_SLOTS)
                    ph0 = h0ps.tile([H, GRP_SLOTS], F32, tag="ph0", padded_shape=[H, 512])
                    nc.tensor.matmul(
                        ph0[:], wt["ne_wT"][:], xres[:, sl], start=True, stop=True
                    )
                    nc.vector.tensor_scalar_add(z_res[:, sl], ph0[:], wt["ne_b"][:])

            if STOP_AFTER == "h0":
                stop_now[0] = True

            # layernorm(channel-major) + affine + relu: dst = relu(g*ln(src)+b)
            def ln_relu(pool, psln, src_ap, dst_ap, C, gamma, beta):
                sq = pool.tile([C, GRP_SLOTS], F32, tag="ln_sq", name="ln_sq")
                nc.scalar.activation(sq[:], src_ap, AF.Square)
                pst = psln.tile([1, 2 * GRP_SLOTS], F32, tag="ln_st", name="ln_st", padded_shape=[1, 512])
                nc.tensor.matmul(
                    pst[:, 0:GRP_SLOTS], ones128[0:C, :], src_ap, start=True, stop=False
                )
                nc.tensor.matmul(
                    pst[:, GRP_SLOTS:], ones128[0:C, :], sq[:], start=False, stop=True
                )
                st = pool.tile([1, 3 * GRP_SLOTS], F32, tag="ln_stats", name="ln_stats")
                mu = st[:, 0:GRP_SLOTS]
                t2 = st[:, GRP_SLOTS : 2 * GRP_SLOTS]
                ab = st[:, 2 * GRP_SLOTS :]
                nc.vector.tensor_scalar_mul(mu, pst[:, 0:GRP_SLOTS], 1.0 / C)
                nc.vector.tensor_scalar(
                    t2, pst[:, GRP_SLOTS:], 1.0 / C, 1e-5, OP.mult, OP.add
                )
                mm = pool.tile([1, GRP_SLOTS], F32, tag="ln_mm", name="ln_mm")
                nc.vector.tensor_tensor(mm[:], mu, mu, OP.mult)
                nc.vector.tensor_tensor(t2, t2, mm[:], OP.subtract)
                nc.vector.reciprocal(t2, t2)
                nc.scalar.activation(ab, t2, AF.Sqrt)
                bv = pool.tile([1, GRP_SLOTS], F32, tag="ln_bv", name="ln_bv")
                nc.vector.scalar_tensor_tensor(bv[:], mu, -1.0, ab, OP.mult, OP.mult)
                prep = psln.tile([128, 2 * GRP_SLOTS], F32, tag="ln_rep", name="ln_rep", padded_shape=[128, 512])
                nc.tensor.matmul(
                    prep[:, 0:GRP_SLOTS], ones1x128[:], ab, start=True, stop=False
                )
                nc.tensor.matmul(
                    prep[:, GRP_SLOTS:], ones1x128[:], bv[:], start=False, stop=True
                )
                zt = pool.tile([C, GRP_SLOTS], F32, tag="ln_zt", name="ln_zt")
                nc.vector.tensor_tensor(zt[:], src_ap, prep[0:C, 0:GRP_SLOTS], OP.mult)
                nc.vector.tensor_tensor(zt[:], zt[:], prep[0:C, GRP_SLOTS:], OP.add)
                nc.scalar.activation(dst_ap, zt[:], AF.Relu, bias=beta, scale=gamma)

            # ============ layers ============
            for l in range(L if not stop_now[0] else 0):
                with (
                    tc.tile_pool(name=f"tpps{l}", bufs=2, space="PSUM") as tpps,
                    tc.tile_pool(name=f"tpsb{l}", bufs=1) as tpsb,
                ):
                    stag = tpsb.tile(
                        [128, n_groups, KCH, H], F32, tag="stag", name="stag"
                    )
                    for g in range(n_groups):
                        for k in range(KCH):
                            s0 = g * GRP_SLOTS + k * 128
                            pt = tpps.tile([128, H], F32, tag="pt", name="pt", padded_shape=[128, 512])
                            nc.tensor.transpose(
                                pt[:], z_res[:, s0 : s0 + 128], wt["eye"][0:H, 0:H]
                            )
                            nc.vector.tensor_copy(stag[:, g, k, :], pt[:])
                    nc.sync.dma_start(
                        cc_in[l].ap().rearrange("(g k p) h -> p g k h", p=128, k=KCH),
                        stag[:],
                    )
                nc.gpsimd.collective_compute(
                    "AllGather",
                    OP.bypass,
                    replica_groups=rg,
                    ins=[cc_in[l].ap()],
                    outs=[tables[l].ap()],
                )
                if STOP_AFTER == "table0" and l == 0:
                    stop_now[0] = True
                    break
                tbl_lo = tables[l].ap()[0 : gp.lo_rows, :]
                tbl_hi = tables[l].ap()[gp.hi_base : s_global, :]

                with (
                    tc.tile_pool(name=f"edge{l}", bufs=2) as ep,
                    tc.tile_pool(name=f"node{l}", bufs=2) as npo,
                    tc.tile_pool(name=f"psg{l}", bufs=2, space="PSUM") as psg,
                    tc.tile_pool(name=f"psp{l}", bufs=2, space="PSUM") as psp,
                    tc.tile_pool(name=f"psln{l}", bufs=1, space="PSUM") as psln,
                ):
                    for g in range(n_groups):
                        pgrp = psg.tile([128, GRP_SLOTS], F32, tag="pgrp", name="pgrp", padded_shape=[128, 512])
                        for half in range(2):
                            nidx = N_LO if half == 0 else N_HI
                            nb = BLK_LO if half == 0 else BLK_HI
                            e0 = g * GRP_EDGES + (0 if half == 0 else N_LO)
                            b0 = e0 // BLK
                            tbl = tbl_lo if half == 0 else tbl_hi
                            y = ep.tile([128, nb, H], F32, tag=f"y{half}", name="y")
                            nc.gpsimd.dma_gather(
                                y[:],
                                tbl,
                                idx_t[:, e0 // 16 : (e0 + nidx) // 16],
                                nidx,
                                nidx,
                                H,
                                single_packet=False,
                            )
                            et = ep.tile([128, nb, H], F32, tag=f"e{half}", name="et")
                            nc.sync.dma_start(et[:], ea_r[:, b0 : b0 + nb, :])
                            nc.vector.tensor_tensor(y[:], y[:], et[:], OP.add)
                            nc.scalar.activation(
                                et[:], y[:], AF.Exp, scale=tconst[l][:]
                            )
                            wv = ep.tile(
                                [128, nb, 2 * H], BF16, tag=f"wv{half}", name="wv"
                            )
                            nc.scalar.activation(
                                wv[:, :, 0:H], et[:], AF.Relu, bias=neg1[:]
                            )
                            nc.vector.scalar_tensor_tensor(
                                wv[:, :, H:], y[:], 0.0, et[:], OP.max, OP.mult
                            )
                            st = ep.tile(
                                [128, nb, W_SLOTS], BF16, tag=f"s{half}", name="st"
                            )
                            nc.sync.dma_start(
                                st[:],
                                din["S"].ap().rearrange("b p w -> p b w")[
                                    :, b0 : b0 + nb, :
                                ],
                            )
                            tpw = T_LO if half == 0 else T_HI
                            for b in range(nb):
                                wl = b // tpw
                                nc.tensor.matmul(
                                    pgrp[:, wl * W_SLOTS : (wl + 1) * W_SLOTS],
                                    wv[:, b, :],
                                    st[:, b, :],
                                    start=(half == 0 and b == 0),
                                    stop=False,
                                )
                        deg_g = npo.tile([1, GRP_SLOTS], BF16, tag="deg_g", name="deg_g")
                        nc.sync.dma_start(
                            deg_g[:],
                            din["degb"].ap()[:, g * GRP_SLOTS : (g + 1) * GRP_SLOTS],
                        )
                        nc.tensor.matmul(
                            pgrp[:],
                            ones1x64b[:],
                            deg_g[:],
                            start=False,
                            stop=True,
                        )
                        sl = slice(g * GRP_SLOTS, (g + 1) * GRP_SLOTS)
                        rec = npo.tile([H, GRP_SLOTS], F32, tag="rec", name="rec")
                        nc.vector.reciprocal(rec[:], pgrp[0:H, :])
                        o65 = npo.tile([H + 1, GRP_SLOTS], F32, tag="o65", name="o65")
                        nc.vector.tensor_tensor(o65[0:H, :], pgrp[H:, :], rec[:], OP.mult)
                        nc.vector.tensor_tensor(
                            o65[0:H, :], o65[0:H, :], z_res[:, sl], OP.add
                        )
                        nc.vector.memset(o65[H : H + 1, :], 1.0)
                        ph1 = psp.tile([HID, GRP_SLOTS], F32, tag="ph1", name="ph1", padded_shape=[HID, 512])
                        nc.tensor.matmul(
                            ph1[:], wt[f"w1b{l}"][:], o65[:], start=True, stop=True
                        )
                        h1s = npo.tile([HID, GRP_SLOTS], F32, tag="h1s", name="h1s")
                        nc.vector.tensor_copy(h1s[:], ph1[:])
                        z1s = npo.tile([HID, GRP_SLOTS], F32, tag="z1s", name="z1s")
                        ln_relu(
                            npo, psln, h1s[:], z1s[:], HID,
                            wt[f"g_{l}"][:], wt[f"bn_{l}"][:],
                        )
                        ph2 = psp.tile([H, GRP_SLOTS], F32, tag="ph2", name="ph2", padded_shape=[H, 512])
                        nc.tensor.matmul(
                            ph2[:], wt[f"w2T{l}"][:], z1s[:], start=True, stop=True
                        )
                        if l == 0:
                            nc.vector.tensor_scalar_add(
                                h_res[:, sl], ph2[:], wt["b2_0"][:]
                            )
                        else:
                            tmp = npo.tile([H, GRP_SLOTS], F32, tag="tmp", name="tmp")
                            nc.vector.tensor_scalar_add(
                                tmp[:], ph2[:], wt[f"b2_{l}"][:]
                            )
                            nc.vector.tensor_tensor(
                                h_res[:, sl], h_res[:, sl], tmp[:], OP.add
                            )
                        if l < L - 1:
                            ln_relu(
                                npo, psln, h_res[:, sl], z_res[:, sl], H,
                                wt[f"lng{l+1}"][:], wt[f"lnb{l+1}"][:],
                            )
                        else:
                            qt = npo.tile([H, GRP_SLOTS], F32, tag="qt", name="qt")
                            ln_relu(
                                npo, psln, h_res[:, sl], qt[:], H,
                                wt["lng0"][:], wt["lnb0"][:],
                            )
                            smask_g = npo.tile(
                                [H, GRP_SLOTS], F32, tag="smask_g", name="smask_g"
                            )
                            nc.sync.dma_start(smask_g[:], din["smask"].ap()[:, sl])
                            nc.vector.tensor_tensor(
                                qt[:], qt[:], smask_g[:], OP.add
                            )
                            nc.vector.reduce_max(
                                winmax[:, g * GRP_WIN : (g + 1) * GRP_WIN],
                                qt[:].rearrange("c (w s) -> c w s", s=W_SLOTS),
                                AX.X,
                                op=OP.max,
                            )

                if STOP_AFTER == f"layer{l}":
                    stop_now[0] = True
                    break

            # ============ readout ============
            if stop_now[0]:
                dummy_out()
            with (
                tc.tile_pool(name="ro", bufs=2) as rop,
                tc.tile_pool(name="rops", bufs=1, space="PSUM") as rops,
            ):
              if not stop_now[0]:
                  gm = rop.tile([H, N_GRAPHS, w_prog], F32, tag="gm", name="gm")
                  nc.sync.dma_start(gm[:], din["gmask"].ap())
                  pooled = rop.tile([H, N_GRAPHS], F32, tag="pooled", name="pooled")
                  for gph in range(N_GRAPHS):
                      tmpm = rop.tile([H, w_prog], F32, tag="tmpm", name="tmpm")
                      nc.vector.tensor_tensor(
                          tmpm[:], winmax[:], gm[:, gph, :], OP.add
                      )
                      nc.vector.reduce_max(
                          pooled[:, gph : gph + 1], tmpm[:], AX.X, op=OP.max
                      )
                  nc.sync.dma_start(pool_in.ap(), pooled[:])
                  nc.gpsimd.collective_compute(
                      "AllReduce",
                      OP.max,
                      replica_groups=rg,
                      ins=[pool_in.ap()],
                      outs=[pool_out.ap()],
                  )
                  pool_sb = rop.tile([H, N_GRAPHS], F32, tag="pool_sb", name="pool_sb")
                  nc.sync.dma_start(pool_sb[:], pool_out.ap())
                  pr = rops.tile([N_GRAPHS, 1], F32, tag="pr", name="pr", padded_shape=[N_GRAPHS, 512])
                  nc.tensor.matmul(
                      pr[:], pool_sb[:], wt["ro_w"][:], start=True, stop=True
                  )
                  res_sb = rop.tile([N_GRAPHS, 1], F32, tag="res_sb", name="res_sb")
                  nc.scalar.activation(res_sb[:], pr[:], AF.Sigmoid, bias=wt["ro_b"][:])
                  nc.sync.dma_start(out_d.ap(), res_sb[:])

    nc.compile()
    return nc


def kernel(**inputs):
    edge_index = np.asarray(inputs["edge_index"])
    batch = np.asarray(inputs["batch"])
    n_nodes = np.asarray(inputs["x"]).shape[0]
    gp = build_plan(edge_index, batch, n_nodes)
    core_maps = build_core_inputs(gp, inputs)
    nc = build_nc(gp, np.asarray(inputs["conv_t"], np.float32))
    res = run_bass_kernel_spmd(nc, core_maps, core_ids=list(range(NC)))
    return np.asarray(res.results[0]["out"], np.float32)



# revision 14
# speedup vs baseline: 1.4134x; 1.4134x over previous
"""DeepGCN (GENConv x3, softmax aggregation) on 8 Trainium2 NeuronCores.

Strategy (edge-parallel, dst-sharded):
  - Nodes are dst-sharded across 8 cores; every edge lives on the core owning
    its dst, so segment-softmax stats need no cross-core combine.
  - Softmax aggregation without segment_max (mathematically identical since
    softmax is shift-invariant and msg >= 0):
        msg = relu(y), w = exp(t*msg) = max(exp(t*y), 1), v = msg*w = relu(y)*exp(t*y)
        agg = seg_sum(v) / seg_sum(w)
    seg_sum via TensorE: per 128-edge block, matmul lhsT=[w-1 | v] (128x128
    bf16) against one-hot dst-slot indicators S (128x32 bf16), accumulating
    into a PSUM group window; the "-1" is fixed by accumulating +deg via a
    K=1 matmul of the per-slot in-degree.
  - Node-level tensors live in per-core "slot space" (windows of 32 slots,
    <=768 edges and one graph per window) so the program is identical on all
    cores (SPMD); all per-core variation is input data.
  - x[src] fetched with dma_gather (256B rows) from a replicated slot-space
    table; the int16 index limit is handled with two overlapping table views
    (lo/hi) plus per-window exact lo/hi section balancing using edges whose
    src slot both views reach.
  - GENConv MLP + LayerNorms run channel-major: partition reductions via
    ones-matmuls, per-node affine replicated via K=1 matmuls.
  - Readout: per-window masked max, per-graph max via additive -inf masks,
    AllReduce(max), sigmoid(pooled @ ro_w + ro_b).
"""

import sys

sys.path.insert(0, "/opt/trn_rl_repo")

import numpy as np
import ml_dtypes

import concourse.bass as bass
import concourse.bacc as bacc
import concourse.mybir as mybir
import concourse.tile as tile
from concourse.bass_utils import run_bass_kernel_spmd

F32 = mybir.dt.float32
BF16 = mybir.dt.bfloat16
I16 = mybir.dt.int16
AF = mybir.ActivationFunctionType
OP = mybir.AluOpType
AX = mybir.AxisListType

NC = 8
H = 64
F_NODE = 128
F_EDGE = 32
HID = 128
L = 3
NEG_BIG = -1.0e30
N_GRAPHS = 64

W_SLOTS = 32          # slots (nodes) per window
WIN_BLOCKS = 6        # 128-edge blocks per window
T_LO = 3              # lo-section blocks per window
T_HI = WIN_BLOCKS - T_LO
BLK = 128
WIN_EDGES = WIN_BLOCKS * BLK          # 768
SEC_LO = T_LO * BLK                   # 384
SEC_HI = T_HI * BLK
GRP_WIN = 8           # windows per PSUM group
GRP_SLOTS = GRP_WIN * W_SLOTS         # 256
GRP_EDGES = GRP_WIN * WIN_EDGES       # 6144
GATHER_LIMIT = 32768  # int16 gather index reach
STOP_AFTER = None  # debug: 'ea','h0','table0','edges0','layer0','layer1','layer2'


class Plan:
    pass


class _StopBuild(Exception):
    pass


import contextlib


@contextlib.contextmanager
def _nullcm():
    yield


# ----------------------------------------------------------------------------
# host-side planning
# ----------------------------------------------------------------------------

def build_plan(edge_index, batch, n_nodes):
    src = edge_index[0].astype(np.int64)
    dst = edge_index[1].astype(np.int64)
    npc = n_nodes // NC

    deg = np.bincount(dst, minlength=n_nodes)

    def pack(core):
        wins = []
        cur = []
        cur_e = 0
        for n in range(core * npc, (core + 1) * npc):
            d = int(deg[n])
            if cur and (
                cur_e + d > WIN_EDGES
                or len(cur) >= W_SLOTS
                or batch[n] != batch[cur[0]]
            ):
                wins.append(cur)
                cur, cur_e = [], 0
            cur.append(n)
            cur_e += d
        if cur:
            wins.append(cur)
        return wins

    core_wins = [pack(c) for c in range(NC)]

    w_prog = max(len(w) for w in core_wins)
    w_prog = ((w_prog + GRP_WIN - 1) // GRP_WIN) * GRP_WIN
    s_core = w_prog * W_SLOTS
    s_global = s_core * NC
    lo_rows = min(s_global, GATHER_LIMIT)
    hi_base = max(0, s_global - GATHER_LIMIT)
    hi_rows = s_global - hi_base
    assert hi_rows <= GATHER_LIMIT and lo_rows <= GATHER_LIMIT, (
        f"slot space too large: {s_global}"
    )

    slot_of = np.full(n_nodes, -1, np.int64)
    win_of_node = np.full(n_nodes, -1, np.int64)
    graph_of_win = np.full((NC, w_prog), -1, np.int64)
    for c in range(NC):
        for w, nodes in enumerate(core_wins[c]):
            base = c * s_core + w * W_SLOTS
            for j, n in enumerate(nodes):
                slot_of[n] = base + j
                win_of_node[n] = w
            graph_of_win[c, w] = batch[nodes[0]]

    src_slot = slot_of[src]
    assert (src_slot >= 0).all()
    edge_win = win_of_node[dst]
    core_of = dst // npc

    e_prog = w_prog * WIN_EDGES
    n_groups = w_prog // GRP_WIN

    plans = []
    for c in range(NC):
        e_ids = np.nonzero(core_of == c)[0]
        win_edges = [[] for _ in range(w_prog)]
        for e in e_ids:
            win_edges[edge_win[e]].append(e)

        perm = np.full(e_prog, -1, np.int64)
        scol = np.full(e_prog, -1, np.int64)
        gidx = np.zeros(e_prog, np.int64)

        for w in range(w_prog):
            ew = np.array(win_edges[w], np.int64)
            base = w * WIN_EDGES
            if not ew.size:
                continue
            ss = src_slot[ew]
            f_lo = ss < hi_base
            f_hi = ss >= lo_rows
            flex = ~(f_lo | f_hi)
            n_t, n_fl, n_fh = ew.size, int(f_lo.sum()), int(f_hi.sum())
            assert n_t <= WIN_EDGES
            assert n_fl <= SEC_LO, f"lo overflow c{c} w{w}: {n_fl}"
            assert n_fh <= SEC_HI, f"hi overflow c{c} w{w}: {n_fh}"
            n_lo = min(SEC_LO, n_t - n_fh)
            lo_ids = np.concatenate([ew[f_lo], ew[flex][: n_lo - n_fl]])
            hi_ids = np.concatenate([ew[flex][n_lo - n_fl :], ew[f_hi]])
            assert lo_ids.size == n_lo and hi_ids.size == n_t - n_lo <= SEC_HI
            lo_ids = lo_ids[np.argsort(src_slot[lo_ids], kind="stable")]
            hi_ids = hi_ids[np.argsort(src_slot[hi_ids], kind="stable")]
            wbase = c * s_core + w * W_SLOTS
            perm[base : base + n_lo] = lo_ids
            scol[base : base + n_lo] = slot_of[dst[lo_ids]] - wbase
            gidx[base : base + n_lo] = src_slot[lo_ids]
            hb = base + SEC_LO
            perm[hb : hb + hi_ids.size] = hi_ids
            scol[hb : hb + hi_ids.size] = slot_of[dst[hi_ids]] - wbase
            gidx[hb : hb + hi_ids.size] = src_slot[hi_ids] - hi_base

        # stream order: per group, the 8 windows' lo sections, then hi sections
        pos = np.arange(e_prog)
        w_all = pos // WIN_EDGES
        off = pos % WIN_EDGES
        g = w_all // GRP_WIN
        wl = w_all % GRP_WIN
        is_lo = off < SEC_LO
        new_pos = np.where(
            is_lo,
            g * GRP_EDGES + wl * SEC_LO + off,
            g * GRP_EDGES + GRP_WIN * SEC_LO + wl * SEC_HI + (off - SEC_LO),
        )
        p = Plan()
        p.perm = np.full(e_prog, -1, np.int64)
        p.scol = np.full(e_prog, -1, np.int64)
        p.gidx = np.zeros(e_prog, np.int64)
        p.perm[new_pos] = perm
        p.scol[new_pos] = scol
        p.gidx[new_pos] = gidx
        plans.append(p)

    g = Plan()
    g.w_prog, g.s_core, g.s_global = w_prog, s_core, s_global
    g.lo_rows, g.hi_base, g.hi_rows = lo_rows, hi_base, hi_rows
    g.e_prog, g.n_groups, g.npc = e_prog, n_groups, npc
    g.slot_of, g.deg = slot_of, deg
    g.core_wins, g.graph_of_win = core_wins, graph_of_win
    g.plans = plans
    g.n_nodes = n_nodes
    return g


def wrap_idx(idx_flat):
    n = idx_flat.shape[0]
    arr = np.zeros((128, n // 16), np.int16)
    arr[np.arange(n) % 16, np.arange(n) // 16] = idx_flat.astype(np.int16)
    for r in range(1, 8):
        arr[16 * r : 16 * (r + 1)] = arr[0:16]
    return arr


def build_core_inputs(gp, inputs):
    x = np.asarray(inputs["x"], np.float32)
    edge_attr = np.asarray(inputs["edge_attr"], np.float32)

    shared = {
        "ne_wT": np.ascontiguousarray(np.asarray(inputs["ne_w"], np.float32).T),
        "ne_b": np.asarray(inputs["ne_b"], np.float32).reshape(H, 1),
        "eaw1b": np.concatenate(
            [
                np.asarray(inputs["ee_w1"], np.float32).T,
                np.asarray(inputs["ee_b1"], np.float32).reshape(1, -1),
            ]
        ).astype(ml_dtypes.bfloat16),
        "eaw2b": np.concatenate(
            [
                np.asarray(inputs["ee_w2"], np.float32).T,
                np.asarray(inputs["ee_b2"], np.float32).reshape(1, -1),
            ]
        ).astype(ml_dtypes.bfloat16),
        "eye": np.eye(128, dtype=np.float32),
        "ro_w": np.ascontiguousarray(
            np.asarray(inputs["ro_w"], np.float32).reshape(1, H).T
        ),
        "ro_b": np.full(
            (N_GRAPHS, 1), float(np.asarray(inputs["ro_b"]).reshape(-1)[0]), np.float32
        ),
    }
    for l in range(L):
        shared[f"w1b{l}"] = np.concatenate(
            [
                np.asarray(inputs["conv_w1"], np.float32)[l].T,
                np.asarray(inputs["conv_b1"], np.float32)[l].reshape(1, -1),
            ]
        )
        shared[f"w2T{l}"] = np.ascontiguousarray(
            np.asarray(inputs["conv_w2"], np.float32)[l].T
        )
        shared[f"b2_{l}"] = np.asarray(inputs["conv_b2"], np.float32)[l].reshape(H, 1)
        shared[f"g_{l}"] = np.asarray(inputs["conv_g"], np.float32)[l].reshape(HID, 1)
        shared[f"bn_{l}"] = np.asarray(inputs["conv_bn"], np.float32)[l].reshape(HID, 1)
        shared[f"lng{l}"] = np.asarray(inputs["ln_g"], np.float32)[l].reshape(H, 1)
        shared[f"lnb{l}"] = np.asarray(inputs["ln_b"], np.float32)[l].reshape(H, 1)

    core_maps = []
    n_lo = GRP_WIN * SEC_LO
    for c in range(NC):
        p = gp.plans[c]
        valid = p.perm >= 0
        perm_safe = np.where(valid, p.perm, 0)

        idx_arr = np.zeros((128, gp.e_prog // 16), np.int16)
        for grp in range(gp.n_groups):
            b0 = grp * GRP_EDGES
            idx_arr[:, b0 // 16 : (b0 + n_lo) // 16] = wrap_idx(
                p.gidx[b0 : b0 + n_lo]
            )
            idx_arr[:, (b0 + n_lo) // 16 : (b0 + GRP_EDGES) // 16] = wrap_idx(
                p.gidx[b0 + n_lo : b0 + GRP_EDGES]
            )

        S = np.zeros((gp.e_prog, W_SLOTS), np.float32)
        pos = np.arange(gp.e_prog)
        S[pos[valid], p.scol[valid]] = 1.0
        S = S.reshape(gp.e_prog // BLK, BLK, W_SLOTS).astype(ml_dtypes.bfloat16)

        ea = edge_attr[perm_safe] * valid[:, None]
        eaT33 = np.concatenate(
            [ea.T, np.ones((1, gp.e_prog), np.float32)]
        ).astype(ml_dtypes.bfloat16)

        node_of_slot = np.full(gp.s_core, -1, np.int64)
        for w, nodes in enumerate(gp.core_wins[c]):
            for j, n in enumerate(nodes):
                node_of_slot[w * W_SLOTS + j] = n
        sv = node_of_slot >= 0
        ns = np.where(sv, node_of_slot, 0)
        degp = np.where(sv, np.maximum(gp.deg[ns], 1), 1).astype(np.float32)

        M = np.full((N_GRAPHS, gp.w_prog), NEG_BIG, np.float32)
        for w in range(gp.w_prog):
            gw = gp.graph_of_win[c, w]
            if gw >= 0:
                M[gw, w] = 0.0

        m = dict(shared)
        m.update(
            {
                "idx": idx_arr,
                "S": S,
                "eaT33": eaT33,
                "degb": degp[None, :].astype(ml_dtypes.bfloat16),
                "xT": np.ascontiguousarray(x[ns].T),
                "smask": np.repeat(
                    np.where(sv, 0.0, NEG_BIG).astype(np.float32)[None, :], H, 0
                ),
                "gmask": np.broadcast_to(
                    M[None], (H, N_GRAPHS, gp.w_prog)
                ).copy(),
            }
        )
        core_maps.append(m)
    return core_maps


# ----------------------------------------------------------------------------
# numpy emulation of the device algorithm (validation aid)
# ----------------------------------------------------------------------------

def numpy_forward(gp, core_maps, inputs):
    conv_t = np.asarray(inputs["conv_t"], np.float32)
    bf = lambda a: a.astype(ml_dtypes.bfloat16).astype(np.float32)

    def ln_cm(h, g, b):
        C = h.shape[0]
        mu = h.mean(0, keepdims=True)
        var = (h * h).mean(0, keepdims=True) - mu * mu
        return (h - mu) / np.sqrt(var + 1e-5) * g + b

    z, h, eas = [], [], []
    n_lo = GRP_WIN * SEC_LO
    for c in range(NC):
        m = core_maps[c]
        z.append(m["ne_wT"].T @ m["xT"] + m["ne_b"])
        h.append(np.zeros_like(z[-1]))
        w1 = m["eaw1b"].astype(np.float32)
        w2 = m["eaw2b"].astype(np.float32)
        ea33 = m["eaT33"].astype(np.float32)
        r1 = bf(np.maximum(w1.T @ ea33, 0))
        r1 = np.concatenate([r1, np.ones((1, gp.e_prog), np.float32)])
        eas.append(bf(w2.T @ r1).T.copy())

    for l in range(L):
        t = float(conv_t[l])
        table = np.concatenate([zz.T for zz in z], axis=0)
        for c in range(NC):
            m = core_maps[c]
            p = gp.plans[c]
            src_rows = np.zeros(gp.e_prog, np.int64)
            for grp in range(gp.n_groups):
                b0 = grp * GRP_EDGES
                src_rows[b0 : b0 + n_lo] = p.gidx[b0 : b0 + n_lo]
                src_rows[b0 + n_lo : b0 + GRP_EDGES] = (
                    p.gidx[b0 + n_lo : b0 + GRP_EDGES] + gp.hi_base
                )
            y = table[src_rows] + eas[c]
            e = bf(np.exp(t * y))
            w1 = bf(np.maximum(e - 1, 0))
            v = bf(np.maximum(y, 0) * e)
            Sb = m["S"].astype(np.float32)
            wv = bf(np.concatenate([w1, v], 1)).reshape(gp.e_prog // BLK, BLK, 2 * H)
            outb = np.einsum("bek,bew->bkw", wv, Sb)
            P = np.zeros((H, gp.s_core), np.float32)
            Q = np.zeros((H, gp.s_core), np.float32)
            bpg = GRP_EDGES // BLK
            for b in range(gp.e_prog // BLK):
                grp, ib = b // bpg, b % bpg
                wl = ib // T_LO if ib < GRP_WIN * T_LO else (ib - GRP_WIN * T_LO) // T_HI
                s0 = grp * GRP_SLOTS + wl * W_SLOTS
                P[:, s0 : s0 + W_SLOTS] += outb[b, 0:H]
                Q[:, s0 : s0 + W_SLOTS] += outb[b, H:]
            P = P + m["degb"].astype(np.float32)
            out_n = Q / P + z[c]
            h1 = m[f"w1b{l}"].T @ np.concatenate(
                [out_n, np.ones((1, gp.s_core), np.float32)]
            )
            z1 = np.maximum(ln_cm(h1, m[f"g_{l}"], m[f"bn_{l}"]), 0)
            h2 = m[f"w2T{l}"].T @ z1 + m[f"b2_{l}"]
            h[c] = h2 if l == 0 else h[c] + h2
            if l < L - 1:
                z[c] = np.maximum(ln_cm(h[c], m[f"lng{l+1}"], m[f"lnb{l+1}"]), 0)

    pooled = np.full((H, N_GRAPHS), NEG_BIG, np.float32)
    for c in range(NC):
        m = core_maps[c]
        q = np.maximum(ln_cm(h[c], m["lng0"], m["lnb0"]), 0) + m["smask"]
        winmax = q.reshape(H, gp.w_prog, W_SLOTS).max(2)
        for gph in range(N_GRAPHS):
            pooled[:, gph] = np.maximum(
                pooled[:, gph], (winmax + m["gmask"][:, gph, :]).max(1)
            )
    r = pooled.T @ core_maps[0]["ro_w"] + core_maps[0]["ro_b"]
    return 1.0 / (1.0 + np.exp(-r))


# ----------------------------------------------------------------------------
# bass program
# ----------------------------------------------------------------------------

def build_nc(gp, conv_t):
    nc = bacc.Bacc(
        "TRN2",
        debug=False,
        num_devices=NC,
        target_bir_lowering=False,
        num_swdge_queues=4,
    )

    e_prog, s_core, s_global = gp.e_prog, gp.s_core, gp.s_global
    n_groups, w_prog = gp.n_groups, gp.w_prog
    nblk = e_prog // BLK
    N_LO, N_HI = GRP_WIN * SEC_LO, GRP_WIN * SEC_HI
    BLK_LO, BLK_HI = GRP_WIN * T_LO, GRP_WIN * T_HI
    KCH = GRP_SLOTS // 128  # 128-col chunks per group (= 2)

    din = {}
    din_dt = {}

    def inp(name, shape, dt=F32):
        din[name] = nc.dram_tensor(name, list(shape), dt, kind="ExternalInput")
        din_dt[name] = dt

    inp("idx", [128, e_prog // 16], I16)
    inp("S", [nblk, BLK, W_SLOTS], BF16)
    inp("eaT33", [F_EDGE + 1, e_prog], BF16)
    inp("degb", [1, s_core], BF16)
    inp("xT", [F_NODE, s_core])
    inp("smask", [H, s_core])
    inp("gmask", [H, N_GRAPHS, w_prog])
    inp("ne_wT", [F_NODE, H])
    inp("ne_b", [H, 1])
    inp("eaw1b", [F_EDGE + 1, F_EDGE], BF16)
    inp("eaw2b", [F_EDGE + 1, H], BF16)
    inp("eye", [128, 128])
    inp("ro_w", [H, 1])
    inp("ro_b", [N_GRAPHS, 1])
    for l in range(L):
        inp(f"w1b{l}", [H + 1, HID])
        inp(f"w2T{l}", [HID, H])
        inp(f"b2_{l}", [H, 1])
        inp(f"g_{l}", [HID, 1])
        inp(f"bn_{l}", [HID, 1])
        inp(f"lng{l}", [H, 1])
        inp(f"lnb{l}", [H, 1])

    out_d = nc.dram_tensor("out", [N_GRAPHS, 1], F32, kind="ExternalOutput")

    ea_dram = nc.dram_tensor("ea_edge", [e_prog, H], BF16, kind="Internal")
    ea_r = ea_dram.ap().rearrange("(b p) h -> p b h", p=128)
    cc_in = [
        nc.dram_tensor(f"cc_in{l}", [s_core, H], F32, kind="Internal")
        for l in range(L)
    ]
    tables = [
        nc.dram_tensor(
            f"table{l}", [s_global, H], F32, kind="Internal", addr_space="Shared"
        )
        for l in range(L)
    ]
    pool_in = nc.dram_tensor("pool_in", [H, N_GRAPHS], F32, kind="Internal")
    pool_out = nc.dram_tensor(
        "pool_out", [H, N_GRAPHS], F32, kind="Internal", addr_space="Shared"
    )
    rg = [list(range(NC))]

    with tile.TileContext(nc) as tc:
        with tc.tile_pool(name="res", bufs=1) as res:
            idx_t = res.tile([128, e_prog // 16], I16)
            nc.sync.dma_start(idx_t[:], din["idx"].ap())
            z_res = res.tile([H, s_core], F32)
            h_res = res.tile([H, s_core], F32)
            winmax = res.tile([H, w_prog], F32)
            degb_sb = res.tile([1, s_core], BF16, name="degb_sb")
            nc.sync.dma_start(degb_sb[:], din["degb"].ap())

            wt = {}
            for name in [
                "ne_wT", "ne_b", "eaw1b", "eaw2b", "eye", "ro_w", "ro_b",
            ] + [
                f"{pre}{l}"
                for l in range(L)
                for pre in ["w1b", "w2T", "b2_", "g_", "bn_", "lng", "lnb"]
            ]:
                wt[name] = res.tile(
                    list(din[name].shape), din_dt[name], name=f"wt_{name}"
                )
                nc.sync.dma_start(wt[name][:], din[name].ap())

            ones1x64b = res.tile([1, 2 * H], BF16)
            nc.vector.memset(ones1x64b[:, 0:H], 1.0)
            nc.vector.memset(ones1x64b[:, H:], 0.0)
            ones128 = res.tile([128, 1], F32)
            nc.vector.memset(ones128[:], 1.0)
            ones1x128 = res.tile([1, 128], F32)
            nc.vector.memset(ones1x128[:], 1.0)
            neg1 = res.tile([128, 1], F32)
            nc.vector.memset(neg1[:], -1.0)
            tconst = []
            for l in range(L):
                ct = res.tile([128, 1], F32, name=f"tconst{l}")
                nc.vector.memset(ct[:], float(conv_t[l]))
                tconst.append(ct)

            # ============ phase 1: edge-attr MLP -> ea_dram (bf16) ============
            CH = 512
            CHB = 2048          # edges per DMA batch (4 x 512 compute chunks)
            NSUB = CHB // CH
            with (
                tc.tile_pool(name="eap", bufs=3) as eap,
                tc.tile_pool(name="eaps", bufs=2, space="PSUM") as eaps,
            ):
                for ch in range(e_prog // CHB):
                    eat = eap.tile([F_EDGE + 1, CHB], BF16, tag="eat")
                    nc.sync.dma_start(
                        eat[:], din["eaT33"].ap()[:, ch * CHB : (ch + 1) * CHB]
                    )
                    eo = eap.tile([128, CHB // 128, H], BF16, tag="eo")
                    for s in range(NSUB):
                        sl = slice(s * CH, (s + 1) * CH)
                        ps1 = eaps.tile([F_EDGE, CH], F32, tag="ps1")
                        nc.tensor.matmul(
                            ps1[:], wt["eaw1b"][:], eat[:, sl], start=True, stop=True
                        )
                        r1 = eap.tile([F_EDGE + 1, CH], BF16, tag="r1")
                        nc.scalar.activation(r1[0:F_EDGE, :], ps1[:], AF.Relu)
                        nc.vector.memset(r1[F_EDGE : F_EDGE + 1, :], 1.0)
                        ps2 = eaps.tile([128, CH // 128, H], F32, tag="ps2", padded_shape=[128, CH // 128, 128])
                        for k in range(CH // 128):
                            nc.tensor.matmul(
                                ps2[:, k, :],
                                r1[:, k * 128 : (k + 1) * 128],
                                wt["eaw2b"][:],
                                start=(k == 0),
                                stop=(k == CH // 128 - 1),
                            )
                        nc.vector.tensor_copy(
                            eo[:, s * (CH // 128) : (s + 1) * (CH // 128), :], ps2[:]
                        )
                    nc.sync.dma_start(
                        ea_r[:, ch * (CHB // 128) : (ch + 1) * (CHB // 128), :], eo[:]
                    )

            stop_now = [False]

            def dummy_out():
                dz = res.tile([N_GRAPHS, 1], F32, name="dummy_out_t")
                nc.vector.memset(dz[:], 0.5)
                nc.sync.dma_start(out_d.ap(), dz[:])

            if STOP_AFTER == "ea":
                stop_now[0] = True
            # ============ phase 2: h0 projection -> z_res ============
            with (
                tc.tile_pool(name="h0sb", bufs=1) as h0sb,
                tc.tile_pool(name="h0ps", bufs=2, space="PSUM") as h0ps,
            ):
                xres = h0sb.tile([F_NODE, s_core], F32, tag="xres")
                nc.sync.dma_start(xres[:], din["xT"].ap())
                for g in range(n_groups):
                    sl = slice(g * GRP_SLOTS, (g + 1) * GRP_SLOTS)
                    ph0 = h0ps.tile([H, GRP_SLOTS], F32, tag="ph0", padded_shape=[H, 512])
                    nc.tensor.matmul(
                        ph0[:], wt["ne_wT"][:], xres[:, sl], start=True, stop=True
                    )
                    nc.vector.tensor_scalar_add(z_res[:, sl], ph0[:], wt["ne_b"][:])

            if STOP_AFTER == "h0":
                stop_now[0] = True

            # layernorm(channel-major) + affine + relu: dst = relu(g*ln(src)+b)
            def ln_relu(pool, psln, src_ap, dst_ap, C, gamma, beta):
                sq = pool.tile([C, GRP_SLOTS], F32, tag="ln_sq", name="ln_sq")
                nc.scalar.activation(sq[:], src_ap, AF.Square)
                pst = psln.tile([1, 2 * GRP_SLOTS], F32, tag="ln_st", name="ln_st", padded_shape=[1, 512])
                nc.tensor.matmul(
                    pst[:, 0:GRP_SLOTS], ones128[0:C, :], src_ap, start=True, stop=False
                )
                nc.tensor.matmul(
                    pst[:, GRP_SLOTS:], ones128[0:C, :], sq[:], start=False, stop=True
                )
                st = pool.tile([1, 3 * GRP_SLOTS], F32, tag="ln_stats", name="ln_stats")
                mu = st[:, 0:GRP_SLOTS]
                t2 = st[:, GRP_SLOTS : 2 * GRP_SLOTS]
                ab = st[:, 2 * GRP_SLOTS :]
                # one fused op over [sum|sumsq]: mu gets a harmless +1e-5 too
                nc.vector.tensor_scalar(
                    st[:, 0 : 2 * GRP_SLOTS], pst[:], 1.0 / C, 1e-5, OP.mult, OP.add
                )
                mm = pool.tile([1, GRP_SLOTS], F32, tag="ln_mm", name="ln_mm")
                nc.vector.tensor_tensor(mm[:], mu, mu, OP.mult)
                nc.vector.tensor_tensor(t2, t2, mm[:], OP.subtract)
                nc.vector.reciprocal(t2, t2)
                nc.scalar.activation(ab, t2, AF.Sqrt)
                bv = pool.tile([1, GRP_SLOTS], F32, tag="ln_bv", name="ln_bv")
                nc.vector.scalar_tensor_tensor(bv[:], mu, -1.0, ab, OP.mult, OP.mult)
                prep = psln.tile([128, 2 * GRP_SLOTS], F32, tag="ln_rep", name="ln_rep", padded_shape=[128, 512])
                nc.tensor.matmul(
                    prep[:, 0:GRP_SLOTS], ones1x128[:], ab, start=True, stop=False
                )
                nc.tensor.matmul(
                    prep[:, GRP_SLOTS:], ones1x128[:], bv[:], start=False, stop=True
                )
                zt = pool.tile([C, GRP_SLOTS], F32, tag="ln_zt", name="ln_zt")
                nc.vector.tensor_tensor(zt[:], src_ap, prep[0:C, 0:GRP_SLOTS], OP.mult)
                nc.vector.tensor_tensor(zt[:], zt[:], prep[0:C, GRP_SLOTS:], OP.add)
                nc.scalar.activation(dst_ap, zt[:], AF.Relu, bias=beta, scale=gamma)

            # ============ layers ============
            for l in range(L if not stop_now[0] else 0):
                with (
                    tc.tile_pool(name=f"tpps{l}", bufs=2, space="PSUM") as tpps,
                    tc.tile_pool(name=f"tpsb{l}", bufs=1) as tpsb,
                ):
                    stag = tpsb.tile(
                        [128, n_groups, KCH, H], F32, tag="stag", name="stag"
                    )
                    for g in range(n_groups):
                        for k in range(KCH):
                            s0 = g * GRP_SLOTS + k * 128
                            pt = tpps.tile([128, H], F32, tag="pt", name="pt", padded_shape=[128, 512])
                            nc.tensor.transpose(
                                pt[:], z_res[:, s0 : s0 + 128], wt["eye"][0:H, 0:H]
                            )
                            nc.vector.tensor_copy(stag[:, g, k, :], pt[:])
                    nc.sync.dma_start(
                        cc_in[l].ap().rearrange("(g k p) h -> p g k h", p=128, k=KCH),
                        stag[:],
                    )
                nc.gpsimd.collective_compute(
                    "AllGather",
                    OP.bypass,
                    replica_groups=rg,
                    ins=[cc_in[l].ap()],
                    outs=[tables[l].ap()],
                )
                if STOP_AFTER == "table0" and l == 0:
                    stop_now[0] = True
                    break
                tbl_lo = tables[l].ap()[0 : gp.lo_rows, :]
                tbl_hi = tables[l].ap()[gp.hi_base : s_global, :]

                with (
                    tc.tile_pool(name=f"edge{l}", bufs=2) as ep,
                    tc.tile_pool(name=f"node{l}", bufs=2) as npo,
                    tc.tile_pool(name=f"psg{l}", bufs=2, space="PSUM") as psg,
                    tc.tile_pool(name=f"psp{l}", bufs=2, space="PSUM") as psp,
                    tc.tile_pool(name=f"psln{l}", bufs=1, space="PSUM") as psln,
                ):
                    gblk = GRP_EDGES // BLK  # 48 blocks per group
                    for g in range(n_groups):
                        b0g = g * gblk
                        et_g = ep.tile([128, gblk, H], BF16, tag="etg", name="etg")
                        nc.sync.dma_start(et_g[:], ea_r[:, b0g : b0g + gblk, :])
                        st_g = ep.tile(
                            [128, gblk, W_SLOTS], BF16, tag="stg", name="stg"
                        )
                        nc.sync.dma_start(
                            st_g[:],
                            din["S"].ap().rearrange("b p w -> p b w")[
                                :, b0g : b0g + gblk, :
                            ],
                        )
                        pgrp = psg.tile([128, GRP_SLOTS], F32, tag="pgrp", name="pgrp", padded_shape=[128, 512])
                        for half in range(2):
                            nidx = N_LO if half == 0 else N_HI
                            nb = BLK_LO if half == 0 else BLK_HI
                            bo = 0 if half == 0 else BLK_LO
                            e0 = g * GRP_EDGES + (0 if half == 0 else N_LO)
                            tbl = tbl_lo if half == 0 else tbl_hi
                            y = ep.tile([128, nb, H], F32, tag=f"y{half}", name="y")
                            nc.gpsimd.dma_gather(
                                y[:],
                                tbl,
                                idx_t[:, e0 // 16 : (e0 + nidx) // 16],
                                nidx,
                                nidx,
                                H,
                                single_packet=False,
                                queue_num=(2 * g + half) % 4,
                            )
                            nc.vector.tensor_tensor(
                                y[:], y[:], et_g[:, bo : bo + nb, :], OP.add
                            )
                            wv = ep.tile(
                                [128, nb, 2 * H], BF16, tag=f"wv{half}", name="wv"
                            )
                            ex = wv[:, :, H:]
                            nc.scalar.activation(ex, y[:], AF.Exp, scale=tconst[l][:])
                            nc.scalar.activation(
                                wv[:, :, 0:H], ex, AF.Relu, bias=neg1[:]
                            )
                            nc.vector.scalar_tensor_tensor(
                                ex, y[:], 0.0, ex, OP.max, OP.mult
                            )
                            tpw = T_LO if half == 0 else T_HI
                            for b in range(nb):
                                wl = b // tpw
                                nc.tensor.matmul(
                                    pgrp[:, wl * W_SLOTS : (wl + 1) * W_SLOTS],
                                    wv[:, b, :],
                                    st_g[:, bo + b, :],
                                    start=(half == 0 and b == 0),
                                    stop=False,
                                )
                        nc.tensor.matmul(
                            pgrp[:],
                            ones1x64b[:],
                            degb_sb[:, g * GRP_SLOTS : (g + 1) * GRP_SLOTS],
                            start=False,
                            stop=True,
                        )
                        sl = slice(g * GRP_SLOTS, (g + 1) * GRP_SLOTS)
                        rec = npo.tile([H, GRP_SLOTS], F32, tag="rec", name="rec")
                        nc.vector.reciprocal(rec[:], pgrp[0:H, :])
                        o65 = npo.tile([H + 1, GRP_SLOTS], F32, tag="o65", name="o65")
                        nc.vector.tensor_tensor(o65[0:H, :], pgrp[H:, :], rec[:], OP.mult)
                        nc.vector.tensor_tensor(
                            o65[0:H, :], o65[0:H, :], z_res[:, sl], OP.add
                        )
                        nc.vector.memset(o65[H : H + 1, :], 1.0)
                        ph1 = psp.tile([HID, GRP_SLOTS], F32, tag="ph1", name="ph1", padded_shape=[HID, 512])
                        nc.tensor.matmul(
                            ph1[:], wt[f"w1b{l}"][:], o65[:], start=True, stop=True
                        )
                        h1s = npo.tile([HID, GRP_SLOTS], F32, tag="h1s", name="h1s")
                        nc.vector.tensor_copy(h1s[:], ph1[:])
                        z1s = npo.tile([HID, GRP_SLOTS], F32, tag="z1s", name="z1s")
                        ln_relu(
                            npo, psln, h1s[:], z1s[:], HID,
                            wt[f"g_{l}"][:], wt[f"bn_{l}"][:],
                        )
                        ph2 = psp.tile([H, GRP_SLOTS], F32, tag="ph2", name="ph2", padded_shape=[H, 512])
                        nc.tensor.matmul(
                            ph2[:], wt[f"w2T{l}"][:], z1s[:], start=True, stop=True
                        )
                        if l == 0:
                            nc.vector.tensor_scalar_add(
                                h_res[:, sl], ph2[:], wt["b2_0"][:]
                            )
                        else:
                            tmp = npo.tile([H, GRP_SLOTS], F32, tag="tmp", name="tmp")
                            nc.vector.tensor_scalar_add(
                                tmp[:], ph2[:], wt[f"b2_{l}"][:]
                            )
                            nc.vector.tensor_tensor(
                                h_res[:, sl], h_res[:, sl], tmp[:], OP.add
                            )
                        if l < L - 1:
                            ln_relu(
                                npo, psln, h_res[:, sl], z_res[:, sl], H,
                                wt[f"lng{l+1}"][:], wt[f"lnb{l+1}"][:],
                            )
                        else:
                            qt = npo.tile([H, GRP_SLOTS], F32, tag="qt", name="qt")
                            ln_relu(
                                npo, psln, h_res[:, sl], qt[:], H,
                                wt["lng0"][:], wt["lnb0"][:],
                            )
                            smask_g = npo.tile(
                                [H, GRP_SLOTS], F32, tag="smask_g", name="smask_g"
                            )
                            nc.sync.dma_start(smask_g[:], din["smask"].ap()[:, sl])
                            nc.vector.tensor_tensor(
                                qt[:], qt[:], smask_g[:], OP.add
                            )
                            nc.vector.reduce_max(
                                winmax[:, g * GRP_WIN : (g + 1) * GRP_WIN],
                                qt[:].rearrange("c (w s) -> c w s", s=W_SLOTS),
                                AX.X,
                                op=OP.max,
                            )

                if STOP_AFTER == f"layer{l}":
                    stop_now[0] = True
                    break

            # ============ readout ============
            if stop_now[0]:
                dummy_out()
            with (
                tc.tile_pool(name="ro", bufs=2) as rop,
                tc.tile_pool(name="rops", bufs=1, space="PSUM") as rops,
            ):
              if not stop_now[0]:
                  gm = rop.tile([H, N_GRAPHS, w_prog], F32, tag="gm", name="gm")
                  nc.sync.dma_start(gm[:], din["gmask"].ap())
                  pooled = rop.tile([H, N_GRAPHS], F32, tag="pooled", name="pooled")
                  for gph in range(N_GRAPHS):
                      tmpm = rop.tile([H, w_prog], F32, tag="tmpm", name="tmpm")
                      nc.vector.tensor_tensor(
                          tmpm[:], winmax[:], gm[:, gph, :], OP.add
                      )
                      nc.vector.reduce_max(
                          pooled[:, gph : gph + 1], tmpm[:], AX.X, op=OP.max
                      )
                  nc.sync.dma_start(pool_in.ap(), pooled[:])
                  nc.gpsimd.collective_compute(
                      "AllReduce",
                      OP.max,
                      replica_groups=rg,
                      ins=[pool_in.ap()],
                      outs=[pool_out.ap()],
                  )
                  pool_sb = rop.tile([H, N_GRAPHS], F32, tag="pool_sb", name="pool_sb")
                  nc.sync.dma_start(pool_sb[:], pool_out.ap())
                  pr = rops.tile([N_GRAPHS, 1], F32, tag="pr", name="pr", padded_shape=[N_GRAPHS, 512])
                  nc.tensor.matmul(
                      pr[:], pool_sb[:], wt["ro_w"][:], start=True, stop=True
                  )
                  res_sb = rop.tile([N_GRAPHS, 1], F32, tag="res_sb", name="res_sb")
                  nc.scalar.activation(res_sb[:], pr[:], AF.Sigmoid, bias=wt["ro_b"][:])
                  nc.sync.dma_start(out_d.ap(), res_sb[:])

    nc.compile()
    return nc


def kernel(**inputs):
    edge_index = np.asarray(inputs["edge_index"])
    batch = np.asarray(inputs["batch"])
    n_nodes = np.asarray(inputs["x"]).shape[0]
    gp = build_plan(edge_index, batch, n_nodes)
    core_maps = build_core_inputs(gp, inputs)
    nc = build_nc(gp, np.asarray(inputs["conv_t"], np.float32))
    res = run_bass_kernel_spmd(nc, core_maps, core_ids=list(range(NC)))
    return np.asarray(res.results[0]["out"], np.float32)



# revision 27
# speedup vs baseline: 1.9131x; 1.3535x over previous
"""DeepGCN (GENConv x3, softmax aggregation) on 8 Trainium2 NeuronCores.

Strategy (edge-parallel, dst-sharded):
  - Nodes are dst-sharded across 8 cores; every edge lives on the core owning
    its dst, so segment-softmax stats need no cross-core combine.
  - Softmax aggregation without segment_max (mathematically identical since
    softmax is shift-invariant and msg >= 0):
        msg = relu(y), w = exp(t*msg) = max(exp(t*y), 1), v = msg*w = relu(y)*exp(t*y)
        agg = seg_sum(v) / seg_sum(w)
    seg_sum via TensorE: per 128-edge block, matmul lhsT=[w-1 | v] (128x128
    bf16) against one-hot dst-slot indicators S (128x32 bf16), accumulating
    into a PSUM group window; the "-1" is fixed by accumulating +deg via a
    K=1 matmul of the per-slot in-degree.
  - Node-level tensors live in per-core "slot space" (windows of 32 slots,
    <=768 edges and one graph per window) so the program is identical on all
    cores (SPMD); all per-core variation is input data.
  - x[src] fetched with dma_gather (256B rows) from a replicated slot-space
    table; the int16 index limit is handled with two overlapping table views
    (lo/hi) plus per-window exact lo/hi section balancing using edges whose
    src slot both views reach.
  - GENConv MLP + LayerNorms run channel-major: partition reductions via
    ones-matmuls, per-node affine replicated via K=1 matmuls.
  - Readout: per-window masked max, per-graph max via additive -inf masks,
    AllReduce(max), sigmoid(pooled @ ro_w + ro_b).
"""

import sys

sys.path.insert(0, "/opt/trn_rl_repo")

import numpy as np
import ml_dtypes

import concourse.bass as bass
import concourse.bacc as bacc
import concourse.mybir as mybir
import concourse.tile as tile
from concourse.bass_utils import run_bass_kernel_spmd

F32 = mybir.dt.float32
BF16 = mybir.dt.bfloat16
I16 = mybir.dt.int16
AF = mybir.ActivationFunctionType
OP = mybir.AluOpType
AX = mybir.AxisListType

NC = 8
H = 64
F_NODE = 128
F_EDGE = 32
HID = 128
L = 3
NEG_BIG = -1.0e30
N_GRAPHS = 64

W_SLOTS = 32          # slots (nodes) per window
WIN_BLOCKS = 6        # 128-edge blocks per window
T_LO = 3              # lo-section blocks per window
T_HI = WIN_BLOCKS - T_LO
BLK = 128
WIN_EDGES = WIN_BLOCKS * BLK          # 768
SEC_LO = T_LO * BLK                   # 384
SEC_HI = T_HI * BLK
GRP_WIN = 8           # windows per PSUM group
GRP_SLOTS = GRP_WIN * W_SLOTS         # 256
GRP_EDGES = GRP_WIN * WIN_EDGES       # 6144
GATHER_LIMIT = 32768  # int16 gather index reach
STOP_AFTER = None  # debug: 'ea','h0','table0','edges0','layer0','layer1','layer2'


class Plan:
    pass


class _StopBuild(Exception):
    pass


import contextlib


@contextlib.contextmanager
def _nullcm():
    yield


# ----------------------------------------------------------------------------
# host-side planning
# ----------------------------------------------------------------------------

def build_plan(edge_index, batch, n_nodes):
    src = edge_index[0].astype(np.int64)
    dst = edge_index[1].astype(np.int64)
    npc = n_nodes // NC

    deg = np.bincount(dst, minlength=n_nodes)

    def pack(core):
        wins = []
        cur = []
        cur_e = 0
        for n in range(core * npc, (core + 1) * npc):
            d = int(deg[n])
            if cur and (
                cur_e + d > WIN_EDGES
                or len(cur) >= W_SLOTS
                or batch[n] != batch[cur[0]]
            ):
                wins.append(cur)
                cur, cur_e = [], 0
            cur.append(n)
            cur_e += d
        if cur:
            wins.append(cur)
        return wins

    core_wins = [pack(c) for c in range(NC)]

    w_prog = max(len(w) for w in core_wins)
    w_prog = ((w_prog + GRP_WIN - 1) // GRP_WIN) * GRP_WIN
    s_core = w_prog * W_SLOTS
    s_global = s_core * NC
    lo_rows = min(s_global, GATHER_LIMIT)
    hi_base = max(0, s_global - GATHER_LIMIT)
    hi_rows = s_global - hi_base
    assert hi_rows <= GATHER_LIMIT and lo_rows <= GATHER_LIMIT, (
        f"slot space too large: {s_global}"
    )

    slot_of = np.full(n_nodes, -1, np.int64)
    win_of_node = np.full(n_nodes, -1, np.int64)
    graph_of_win = np.full((NC, w_prog), -1, np.int64)
    for c in range(NC):
        for w, nodes in enumerate(core_wins[c]):
            base = c * s_core + w * W_SLOTS
            for j, n in enumerate(nodes):
                slot_of[n] = base + j
                win_of_node[n] = w
            graph_of_win[c, w] = batch[nodes[0]]

    src_slot = slot_of[src]
    assert (src_slot >= 0).all()
    edge_win = win_of_node[dst]
    core_of = dst // npc

    e_prog = w_prog * WIN_EDGES
    n_groups = w_prog // GRP_WIN

    plans = []
    for c in range(NC):
        e_ids = np.nonzero(core_of == c)[0]
        win_edges = [[] for _ in range(w_prog)]
        for e in e_ids:
            win_edges[edge_win[e]].append(e)

        perm = np.full(e_prog, -1, np.int64)
        scol = np.full(e_prog, -1, np.int64)
        gidx = np.zeros(e_prog, np.int64)

        for w in range(w_prog):
            ew = np.array(win_edges[w], np.int64)
            base = w * WIN_EDGES
            if not ew.size:
                continue
            ss = src_slot[ew]
            f_lo = ss < hi_base
            f_hi = ss >= lo_rows
            flex = ~(f_lo | f_hi)
            n_t, n_fl, n_fh = ew.size, int(f_lo.sum()), int(f_hi.sum())
            assert n_t <= WIN_EDGES
            assert n_fl <= SEC_LO, f"lo overflow c{c} w{w}: {n_fl}"
            assert n_fh <= SEC_HI, f"hi overflow c{c} w{w}: {n_fh}"
            n_lo = min(SEC_LO, n_t - n_fh)
            lo_ids = np.concatenate([ew[f_lo], ew[flex][: n_lo - n_fl]])
            hi_ids = np.concatenate([ew[flex][n_lo - n_fl :], ew[f_hi]])
            assert lo_ids.size == n_lo and hi_ids.size == n_t - n_lo <= SEC_HI
            lo_ids = lo_ids[np.argsort(src_slot[lo_ids], kind="stable")]
            hi_ids = hi_ids[np.argsort(src_slot[hi_ids], kind="stable")]
            wbase = c * s_core + w * W_SLOTS
            perm[base : base + n_lo] = lo_ids
            scol[base : base + n_lo] = slot_of[dst[lo_ids]] - wbase
            gidx[base : base + n_lo] = src_slot[lo_ids]
            hb = base + SEC_LO
            perm[hb : hb + hi_ids.size] = hi_ids
            scol[hb : hb + hi_ids.size] = slot_of[dst[hi_ids]] - wbase
            gidx[hb : hb + hi_ids.size] = src_slot[hi_ids] - hi_base

        # stream order: per group, the 8 windows' lo sections, then hi sections
        pos = np.arange(e_prog)
        w_all = pos // WIN_EDGES
        off = pos % WIN_EDGES
        g = w_all // GRP_WIN
        wl = w_all % GRP_WIN
        is_lo = off < SEC_LO
        new_pos = np.where(
            is_lo,
            g * GRP_EDGES + wl * SEC_LO + off,
            g * GRP_EDGES + GRP_WIN * SEC_LO + wl * SEC_HI + (off - SEC_LO),
        )
        p = Plan()
        p.perm = np.full(e_prog, -1, np.int64)
        p.scol = np.full(e_prog, -1, np.int64)
        p.gidx = np.zeros(e_prog, np.int64)
        p.perm[new_pos] = perm
        p.scol[new_pos] = scol
        p.gidx[new_pos] = gidx
        plans.append(p)

    g = Plan()
    g.w_prog, g.s_core, g.s_global = w_prog, s_core, s_global
    g.lo_rows, g.hi_base, g.hi_rows = lo_rows, hi_base, hi_rows
    g.e_prog, g.n_groups, g.npc = e_prog, n_groups, npc
    g.slot_of, g.deg = slot_of, deg
    g.core_wins, g.graph_of_win = core_wins, graph_of_win
    g.plans = plans
    g.n_nodes = n_nodes
    return g


def wrap_idx(idx_flat):
    n = idx_flat.shape[0]
    arr = np.zeros((128, n // 16), np.int16)
    arr[np.arange(n) % 16, np.arange(n) // 16] = idx_flat.astype(np.int16)
    for r in range(1, 8):
        arr[16 * r : 16 * (r + 1)] = arr[0:16]
    return arr


def build_core_inputs(gp, inputs):
    x = np.asarray(inputs["x"], np.float32)
    edge_attr = np.asarray(inputs["edge_attr"], np.float32)

    # edge MLP weights with an extra "ones" output channel on layer 1 (W=0,
    # b=1 -> relu gives the constant-1 row that carries b2 through layer 2)
    w1t = np.asarray(inputs["ee_w1"], np.float32).T          # [32, 32]
    b1e = np.asarray(inputs["ee_b1"], np.float32).reshape(1, -1)
    eaw1b = np.block(
        [[w1t, np.zeros((F_EDGE, 1), np.float32)], [b1e, np.ones((1, 1), np.float32)]]
    )                                                         # [33, 33]
    shared = {
        "ne_wT": np.ascontiguousarray(np.asarray(inputs["ne_w"], np.float32).T),
        "ne_b": np.asarray(inputs["ne_b"], np.float32).reshape(H, 1),
        "eaw1b": eaw1b.astype(ml_dtypes.bfloat16),
        "eaw2b": np.concatenate(
            [
                np.asarray(inputs["ee_w2"], np.float32).T,
                np.asarray(inputs["ee_b2"], np.float32).reshape(1, -1),
            ]
        ).astype(ml_dtypes.bfloat16),
        "eye": np.eye(128, dtype=np.float32),
        "ro_w": np.ascontiguousarray(
            np.asarray(inputs["ro_w"], np.float32).reshape(1, H).T
        ),
        "ro_b": np.full(
            (N_GRAPHS, 1), float(np.asarray(inputs["ro_b"]).reshape(-1)[0]), np.float32
        ),
    }
    for l in range(L):
        shared[f"w1b{l}"] = np.ascontiguousarray(
            np.asarray(inputs["conv_w1"], np.float32)[l].T
        )
        shared[f"b1r{l}"] = np.asarray(inputs["conv_b1"], np.float32)[l].reshape(
            1, HID
        )
        shared[f"w2T{l}"] = np.ascontiguousarray(
            np.asarray(inputs["conv_w2"], np.float32)[l].T
        )
        shared[f"b2_{l}"] = np.asarray(inputs["conv_b2"], np.float32)[l].reshape(H, 1)
        shared[f"g_{l}"] = np.asarray(inputs["conv_g"], np.float32)[l].reshape(HID, 1)
        shared[f"bn_{l}"] = np.asarray(inputs["conv_bn"], np.float32)[l].reshape(HID, 1)
        shared[f"lng{l}"] = np.asarray(inputs["ln_g"], np.float32)[l].reshape(H, 1)
        shared[f"lnb{l}"] = np.asarray(inputs["ln_b"], np.float32)[l].reshape(H, 1)

    core_maps = []
    n_lo = GRP_WIN * SEC_LO
    for c in range(NC):
        p = gp.plans[c]
        valid = p.perm >= 0
        perm_safe = np.where(valid, p.perm, 0)

        idx_arr = np.zeros((128, gp.e_prog // 16), np.int16)
        for grp in range(gp.n_groups):
            b0 = grp * GRP_EDGES
            idx_arr[:, b0 // 16 : (b0 + n_lo) // 16] = wrap_idx(
                p.gidx[b0 : b0 + n_lo]
            )
            idx_arr[:, (b0 + n_lo) // 16 : (b0 + GRP_EDGES) // 16] = wrap_idx(
                p.gidx[b0 + n_lo : b0 + GRP_EDGES]
            )

        S = np.zeros((gp.e_prog, W_SLOTS), np.float32)
        pos = np.arange(gp.e_prog)
        S[pos[valid], p.scol[valid]] = 1.0
        S = S.reshape(gp.e_prog // BLK, BLK, W_SLOTS).astype(ml_dtypes.bfloat16)

        ea = edge_attr[perm_safe] * valid[:, None]
        eaT33 = np.concatenate(
            [ea.T, np.ones((1, gp.e_prog), np.float32)]
        ).astype(ml_dtypes.bfloat16)

        node_of_slot = np.full(gp.s_core, -1, np.int64)
        for w, nodes in enumerate(gp.core_wins[c]):
            for j, n in enumerate(nodes):
                node_of_slot[w * W_SLOTS + j] = n
        sv = node_of_slot >= 0
        ns = np.where(sv, node_of_slot, 0)
        degp = np.where(sv, np.maximum(gp.deg[ns], 1), 1).astype(np.float32)

        M = np.full((N_GRAPHS, gp.w_prog), NEG_BIG, np.float32)
        for w in range(gp.w_prog):
            gw = gp.graph_of_win[c, w]
            if gw >= 0:
                M[gw, w] = 0.0

        m = dict(shared)
        m.update(
            {
                "idx": idx_arr,
                "S": S,
                "eaT33": eaT33,
                "degb": degp[None, :].astype(ml_dtypes.bfloat16),
                "xT": np.ascontiguousarray(x[ns].T),
                "smask": np.repeat(
                    np.where(sv, 0.0, NEG_BIG).astype(np.float32)[None, :], H, 0
                ),
                "gmask": np.broadcast_to(
                    M[None], (H, N_GRAPHS, gp.w_prog)
                ).copy(),
            }
        )
        core_maps.append(m)
    return core_maps


# ----------------------------------------------------------------------------
# numpy emulation of the device algorithm (validation aid)
# ----------------------------------------------------------------------------

def numpy_forward(gp, core_maps, inputs):
    conv_t = np.asarray(inputs["conv_t"], np.float32)
    bf = lambda a: a.astype(ml_dtypes.bfloat16).astype(np.float32)

    def ln_cm(h, g, b):
        C = h.shape[0]
        mu = h.mean(0, keepdims=True)
        var = (h * h).mean(0, keepdims=True) - mu * mu
        return (h - mu) / np.sqrt(var + 1e-5) * g + b

    z, h, eas = [], [], []
    n_lo = GRP_WIN * SEC_LO
    for c in range(NC):
        m = core_maps[c]
        z.append(m["ne_wT"].T @ m["xT"] + m["ne_b"])
        h.append(np.zeros_like(z[-1]))
        w1 = m["eaw1b"].astype(np.float32)
        w2 = m["eaw2b"].astype(np.float32)
        ea33 = m["eaT33"].astype(np.float32)
        r1 = bf(np.maximum(w1.T @ ea33, 0))   # [33, E] incl. ones channel
        eas.append(bf(w2.T @ r1).T.copy())

    for l in range(L):
        t = float(conv_t[l])
        table = np.concatenate([zz.T for zz in z], axis=0)
        for c in range(NC):
            m = core_maps[c]
            p = gp.plans[c]
            src_rows = np.zeros(gp.e_prog, np.int64)
            for grp in range(gp.n_groups):
                b0 = grp * GRP_EDGES
                src_rows[b0 : b0 + n_lo] = p.gidx[b0 : b0 + n_lo]
                src_rows[b0 + n_lo : b0 + GRP_EDGES] = (
                    p.gidx[b0 + n_lo : b0 + GRP_EDGES] + gp.hi_base
                )
            y = table[src_rows] + eas[c]
            e = bf(np.exp(t * y))
            w1 = bf(np.maximum(e - 1, 0))
            v = bf(np.maximum(y, 0) * e)
            Sb = m["S"].astype(np.float32)
            wv = bf(np.concatenate([w1, v], 1)).reshape(gp.e_prog // BLK, BLK, 2 * H)
            outb = np.einsum("bek,bew->bkw", wv, Sb)
            P = np.zeros((H, gp.s_core), np.float32)
            Q = np.zeros((H, gp.s_core), np.float32)
            bpg = GRP_EDGES // BLK
            for b in range(gp.e_prog // BLK):
                grp, ib = b // bpg, b % bpg
                wl = ib // T_LO if ib < GRP_WIN * T_LO else (ib - GRP_WIN * T_LO) // T_HI
                s0 = grp * GRP_SLOTS + wl * W_SLOTS
                P[:, s0 : s0 + W_SLOTS] += outb[b, 0:H]
                Q[:, s0 : s0 + W_SLOTS] += outb[b, H:]
            P = P + m["degb"].astype(np.float32)
            out_n = Q / P + z[c]
            h1 = m[f"w1b{l}"].T @ out_n + m[f"b1r{l}"].T
            z1 = np.maximum(ln_cm(h1, m[f"g_{l}"], m[f"bn_{l}"]), 0)
            h2 = m[f"w2T{l}"].T @ z1 + m[f"b2_{l}"]
            h[c] = h2 if l == 0 else h[c] + h2
            if l < L - 1:
                z[c] = np.maximum(ln_cm(h[c], m[f"lng{l+1}"], m[f"lnb{l+1}"]), 0)

    pooled = np.full((H, N_GRAPHS), NEG_BIG, np.float32)
    for c in range(NC):
        m = core_maps[c]
        q = np.maximum(ln_cm(h[c], m["lng0"], m["lnb0"]), 0) + m["smask"]
        winmax = q.reshape(H, gp.w_prog, W_SLOTS).max(2)
        for gph in range(N_GRAPHS):
            pooled[:, gph] = np.maximum(
                pooled[:, gph], (winmax + m["gmask"][:, gph, :]).max(1)
            )
    r = pooled.T @ core_maps[0]["ro_w"] + core_maps[0]["ro_b"]
    return 1.0 / (1.0 + np.exp(-r))


# ----------------------------------------------------------------------------
# bass program
# ----------------------------------------------------------------------------

def build_nc(gp, conv_t):
    nc = bacc.Bacc(
        "TRN2",
        debug=False,
        num_devices=NC,
        target_bir_lowering=False,
        num_swdge_queues=4,
    )

    e_prog, s_core, s_global = gp.e_prog, gp.s_core, gp.s_global
    n_groups, w_prog = gp.n_groups, gp.w_prog
    nblk = e_prog // BLK
    N_LO, N_HI = GRP_WIN * SEC_LO, GRP_WIN * SEC_HI
    BLK_LO, BLK_HI = GRP_WIN * T_LO, GRP_WIN * T_HI
    KCH = GRP_SLOTS // 128  # 128-col chunks per group (= 2)

    din = {}
    din_dt = {}

    def inp(name, shape, dt=F32):
        din[name] = nc.dram_tensor(name, list(shape), dt, kind="ExternalInput")
        din_dt[name] = dt

    inp("idx", [128, e_prog // 16], I16)
    inp("S", [nblk, BLK, W_SLOTS], BF16)
    inp("eaT33", [F_EDGE + 1, e_prog], BF16)
    inp("degb", [1, s_core], BF16)
    inp("xT", [F_NODE, s_core])
    inp("smask", [H, s_core])
    inp("gmask", [H, N_GRAPHS, w_prog])
    inp("ne_wT", [F_NODE, H])
    inp("ne_b", [H, 1])
    inp("eaw1b", [F_EDGE + 1, F_EDGE + 1], BF16)
    inp("eaw2b", [F_EDGE + 1, H], BF16)
    inp("eye", [128, 128])
    inp("ro_w", [H, 1])
    inp("ro_b", [N_GRAPHS, 1])
    for l in range(L):
        inp(f"w1b{l}", [H, HID])
        inp(f"b1r{l}", [1, HID])
        inp(f"w2T{l}", [HID, H])
        inp(f"b2_{l}", [H, 1])
        inp(f"g_{l}", [HID, 1])
        inp(f"bn_{l}", [HID, 1])
        inp(f"lng{l}", [H, 1])
        inp(f"lnb{l}", [H, 1])

    out_d = nc.dram_tensor("out", [N_GRAPHS, 1], F32, kind="ExternalOutput")

    # per-group ea tensors: layer-0 group g depends only on its own MLP chunk
    ea_gr = [
        nc.dram_tensor(f"ea_g{g}", [GRP_EDGES, H], BF16, kind="Internal")
        .ap()
        .rearrange("(b p) h -> p b h", p=128)
        for g in range(n_groups)
    ]
    cc_in = [
        nc.dram_tensor(f"cc_in{l}", [s_core, H], F32, kind="Internal")
        for l in range(L)
    ]
    tables = [
        nc.dram_tensor(
            f"table{l}", [s_global, H], F32, kind="Internal", addr_space="Shared"
        )
        for l in range(L)
    ]
    pool_in = nc.dram_tensor("pool_in", [H, N_GRAPHS], F32, kind="Internal")
    pool_out = nc.dram_tensor(
        "pool_out", [H, N_GRAPHS], F32, kind="Internal", addr_space="Shared"
    )
    rg = [list(range(NC))]

    with tile.TileContext(nc) as tc:
        with tc.tile_pool(name="res", bufs=1) as res:
            idx_t = res.tile([128, e_prog // 16], I16)
            nc.sync.dma_start(idx_t[:], din["idx"].ap())
            z_res = res.tile([H, s_core], F32)
            h_res = res.tile([H, s_core], F32)
            winmax = res.tile([H, w_prog], F32)
            degb_sb = res.tile([1, s_core], BF16, name="degb_sb")
            nc.sync.dma_start(degb_sb[:], din["degb"].ap())

            wt = {}
            for name in [
                "ne_wT", "ne_b", "eaw1b", "eaw2b", "eye", "ro_w", "ro_b",
            ] + [
                f"{pre}{l}"
                for l in range(L)
                for pre in ["w1b", "b1r", "w2T", "b2_", "g_", "bn_", "lng", "lnb"]
            ]:
                wt[name] = res.tile(
                    list(din[name].shape), din_dt[name], name=f"wt_{name}"
                )
                nc.sync.dma_start(wt[name][:], din[name].ap())

            ones1x64b = res.tile([1, 2 * H], BF16)
            nc.vector.memset(ones1x64b[:, 0:H], 1.0)
            nc.vector.memset(ones1x64b[:, H:], 0.0)
            ones128 = res.tile([128, 1], F32)
            nc.vector.memset(ones128[:], 1.0)
            ones1x128 = res.tile([1, 128], F32)
            nc.vector.memset(ones1x128[:], 1.0)
            ones1xG = res.tile([1, GRP_SLOTS], F32)
            nc.vector.memset(ones1xG[:], 1.0)
            neg1 = res.tile([128, 1], F32)
            nc.vector.memset(neg1[:], -1.0)
            tconst = []
            for l in range(L):
                ct = res.tile([128, 1], F32, name=f"tconst{l}")
                nc.vector.memset(ct[:], float(conv_t[l]))
                tconst.append(ct)

            stop_now = [False]

            def dummy_out():
                dz = res.tile([N_GRAPHS, 1], F32, name="dummy_out_t")
                nc.vector.memset(dz[:], 0.5)
                nc.sync.dma_start(out_d.ap(), dz[:])

            # ============ phase 1: h0 projection -> z_res ============
            with (
                tc.tile_pool(name="h0sb", bufs=1) as h0sb,
                tc.tile_pool(name="h0ps", bufs=2, space="PSUM") as h0ps,
            ):
                xres = h0sb.tile([F_NODE, s_core], F32, tag="xres")
                nc.sync.dma_start(xres[:], din["xT"].ap())
                for g in range(n_groups):
                    sl = slice(g * GRP_SLOTS, (g + 1) * GRP_SLOTS)
                    ph0 = h0ps.tile([H, GRP_SLOTS], F32, tag="ph0", padded_shape=[H, 512])
                    nc.tensor.matmul(
                        ph0[:], wt["ne_wT"][:], xres[:, sl], start=True, stop=True
                    )
                    nc.vector.tensor_scalar_add(z_res[:, sl], ph0[:], wt["ne_b"][:])

            if STOP_AFTER == "h0":
                stop_now[0] = True

            # transposes z_res -> slot-major staging -> AllGather table[l]
            def build_table(l):
                with (
                    tc.tile_pool(name=f"tpps{l}", bufs=2, space="PSUM") as tpps,
                    tc.tile_pool(name=f"tpsb{l}", bufs=1) as tpsb,
                ):
                    stag = tpsb.tile(
                        [128, n_groups, KCH, H], F32, tag="stag", name="stag"
                    )
                    for g in range(n_groups):
                        for k in range(KCH):
                            s0 = g * GRP_SLOTS + k * 128
                            pt = tpps.tile([128, H], F32, tag="pt", name="pt", padded_shape=[128, 512])
                            nc.tensor.transpose(
                                pt[:], z_res[:, s0 : s0 + 128], wt["eye"][0:H, 0:H]
                            )
                            nc.vector.tensor_copy(stag[:, g, k, :], pt[:])
                    nc.sync.dma_start(
                        cc_in[l].ap().rearrange("(g k p) h -> p g k h", p=128, k=KCH),
                        stag[:],
                    )
                nc.gpsimd.collective_compute(
                    "AllGather",
                    OP.bypass,
                    replica_groups=rg,
                    ins=[cc_in[l].ap()],
                    outs=[tables[l].ap()],
                )

            if not stop_now[0]:
                build_table(0)
            if STOP_AFTER == "table0":
                stop_now[0] = True

            # ======= phase 2: edge-attr MLP -> per-group ea (bf16) =======
            # overlaps with the table-0 AllGather above; layer-0 group g only
            # waits for its own 3 chunks (per-group dram tensors)
            CH1 = 512           # mm1 moving-operand width (one PSUM bank)
            CHB = 2048          # edges per DMA batch
            GCH = GRP_EDGES // CHB  # 3 chunks per group
            with (
                tc.tile_pool(name="eap", bufs=3) as eap,
                tc.tile_pool(name="eaps", bufs=2, space="PSUM") as eaps,
            ):
              if not stop_now[0]:
                for g in range(n_groups):
                    for c3 in range(GCH):
                        ebase = g * GRP_EDGES + c3 * CHB
                        eat = eap.tile([F_EDGE + 1, CHB], BF16, tag="eat")
                        nc.sync.dma_start(
                            eat[:], din["eaT33"].ap()[:, ebase : ebase + CHB]
                        )
                        eo = eap.tile([128, CHB // 128, H], BF16, tag="eo")
                        for s in range(CHB // CH1):
                            sl = slice(s * CH1, (s + 1) * CH1)
                            ps1 = eaps.tile([F_EDGE + 1, CH1], F32, tag="ps1")
                            nc.tensor.matmul(
                                ps1[:], wt["eaw1b"][:], eat[:, sl], start=True, stop=True
                            )
                            r1 = eap.tile([F_EDGE + 1, CH1], BF16, tag="r1")
                            nc.scalar.activation(r1[:], ps1[:], AF.Relu)
                            ps2 = eaps.tile([128, 4, H], F32, tag="ps2", padded_shape=[128, 4, 128])
                            for k in range(4):
                                nc.tensor.matmul(
                                    ps2[:, k, :],
                                    r1[:, k * 128 : (k + 1) * 128],
                                    wt["eaw2b"][:],
                                    start=(k == 0),
                                    stop=(k == 3),
                                )
                            nc.vector.tensor_copy(
                                eo[:, s * 4 : (s + 1) * 4, :], ps2[:]
                            )
                        nc.sync.dma_start(
                            ea_gr[g][:, c3 * (CHB // 128) : (c3 + 1) * (CHB // 128), :],
                            eo[:],
                        )

            if STOP_AFTER == "ea":
                stop_now[0] = True

            # layernorm(channel-major) + affine + relu: dst = relu(g*ln(src)+b)
            def ln_relu(pool, psln, src_ap, dst_ap, C, gamma, beta):
                sq = pool.tile([C, GRP_SLOTS], F32, tag="ln_sq", name="ln_sq")
                nc.scalar.activation(sq[:], src_ap, AF.Square)
                pst = psln.tile([1, 2 * GRP_SLOTS], F32, tag="ln_st", name="ln_st", padded_shape=[1, 512])
                nc.tensor.matmul(
                    pst[:, 0:GRP_SLOTS], ones128[0:C, :], src_ap, start=True, stop=False
                )
                nc.tensor.matmul(
                    pst[:, GRP_SLOTS:], ones128[0:C, :], sq[:], start=False, stop=True
                )
                st = pool.tile([1, 3 * GRP_SLOTS], F32, tag="ln_stats", name="ln_stats")
                mu = st[:, 0:GRP_SLOTS]
                t2 = st[:, GRP_SLOTS : 2 * GRP_SLOTS]
                ab = st[:, 2 * GRP_SLOTS :]
                # one fused op over [sum|sumsq]: mu gets a harmless +1e-5 too
                nc.vector.tensor_scalar(
                    st[:, 0 : 2 * GRP_SLOTS], pst[:], 1.0 / C, 1e-5, OP.mult, OP.add
                )
                mm = pool.tile([1, GRP_SLOTS], F32, tag="ln_mm", name="ln_mm")
                nc.vector.tensor_tensor(mm[:], mu, mu, OP.mult)
                nc.vector.tensor_tensor(t2, t2, mm[:], OP.subtract)
                nc.vector.reciprocal(t2, t2)
                nc.scalar.activation(ab, t2, AF.Sqrt)
                bv = pool.tile([1, GRP_SLOTS], F32, tag="ln_bv", name="ln_bv")
                nc.vector.scalar_tensor_tensor(bv[:], mu, -1.0, ab, OP.mult, OP.mult)
                prep = psln.tile([128, 2 * GRP_SLOTS], F32, tag="ln_rep", name="ln_rep", padded_shape=[128, 512])
                nc.tensor.matmul(
                    prep[:, 0:GRP_SLOTS], ones1x128[:], ab, start=True, stop=False
                )
                nc.tensor.matmul(
                    prep[:, GRP_SLOTS:], ones1x128[:], bv[:], start=False, stop=True
                )
                zt = pool.tile([C, GRP_SLOTS], F32, tag="ln_zt", name="ln_zt")
                nc.vector.tensor_tensor(zt[:], src_ap, prep[0:C, 0:GRP_SLOTS], OP.mult)
                nc.vector.tensor_tensor(zt[:], zt[:], prep[0:C, GRP_SLOTS:], OP.add)
                nc.scalar.activation(dst_ap, zt[:], AF.Relu, bias=beta, scale=gamma)

            # ============ layers ============
            for l in range(L if not stop_now[0] else 0):
                if l > 0:
                    build_table(l)
                tbl_lo = tables[l].ap()[0 : gp.lo_rows, :]
                tbl_hi = tables[l].ap()[gp.hi_base : s_global, :]

                with (
                    tc.tile_pool(name=f"edge{l}", bufs=2) as ep,
                    tc.tile_pool(name=f"node{l}", bufs=2) as npo,
                    tc.tile_pool(name=f"psg{l}", bufs=2, space="PSUM") as psg,
                    tc.tile_pool(name=f"psp{l}", bufs=2, space="PSUM") as psp,
                    tc.tile_pool(name=f"psln{l}", bufs=1, space="PSUM") as psln,
                ):
                    gblk = GRP_EDGES // BLK   # 48 blocks per group
                    QBLK = gblk // 4          # 12 blocks per quarter
                    QIDX = QBLK * BLK         # 1536 edges per quarter-gather
                    for g in range(n_groups):
                        b0g = g * gblk
                        et_g = ep.tile([128, gblk, H], BF16, tag="etg", name="etg")
                        nc.sync.dma_start(et_g[:], ea_gr[g])
                        st_g = ep.tile(
                            [128, gblk, W_SLOTS], BF16, tag="stg", name="stg"
                        )
                        nc.sync.dma_start(
                            st_g[:],
                            din["S"].ap().rearrange("b p w -> p b w")[
                                :, b0g : b0g + gblk, :
                            ],
                        )
                        pgrp = psg.tile([128, GRP_SLOTS], F32, tag="pgrp", name="pgrp", padded_shape=[128, 512])
                        for q in range(4):
                            half = q // 2
                            bo = q * QBLK
                            e0 = g * GRP_EDGES + q * QIDX
                            tbl = tbl_lo if half == 0 else tbl_hi
                            y = ep.tile([128, QBLK, H], F32, tag=f"y{q}", name="y")
                            nc.gpsimd.dma_gather(
                                y[:],
                                tbl,
                                idx_t[:, e0 // 16 : (e0 + QIDX) // 16],
                                QIDX,
                                QIDX,
                                H,
                                single_packet=False,
                                queue_num=q,
                            )
                            nc.vector.tensor_tensor(
                                y[:], y[:], et_g[:, bo : bo + QBLK, :], OP.add
                            )
                            wv = ep.tile(
                                [128, QBLK, 2 * H], BF16, tag=f"wv{q}", name="wv"
                            )
                            ex = wv[:, :, H:]
                            nc.scalar.activation(ex, y[:], AF.Exp, scale=tconst[l][:])
                            nc.scalar.activation(
                                wv[:, :, 0:H], ex, AF.Relu, bias=neg1[:]
                            )
                            nc.vector.scalar_tensor_tensor(
                                ex, y[:], 0.0, ex, OP.max, OP.mult
                            )
                            tpw = T_LO if half == 0 else T_HI
                            for b in range(QBLK):
                                hb = bo + b - half * BLK_LO  # block within half
                                wl = hb // tpw
                                nc.tensor.matmul(
                                    pgrp[:, wl * W_SLOTS : (wl + 1) * W_SLOTS],
                                    wv[:, b, :],
                                    st_g[:, bo + b, :],
                                    start=(q == 0 and b == 0),
                                    stop=False,
                                )
                        nc.tensor.matmul(
                            pgrp[:],
                            ones1x64b[:],
                            degb_sb[:, g * GRP_SLOTS : (g + 1) * GRP_SLOTS],
                            start=False,
                            stop=True,
                        )
                        sl = slice(g * GRP_SLOTS, (g + 1) * GRP_SLOTS)
                        rec = npo.tile([H, GRP_SLOTS], F32, tag="rec", name="rec")
                        nc.vector.reciprocal(rec[:], pgrp[0:H, :])
                        o65 = npo.tile([H, GRP_SLOTS], F32, tag="o65", name="o65")
                        nc.vector.tensor_tensor(o65[:], pgrp[H:, :], rec[:], OP.mult)
                        nc.vector.tensor_tensor(
                            o65[:], o65[:], z_res[:, sl], OP.add
                        )
                        ph1 = psp.tile([HID, GRP_SLOTS], F32, tag="ph1", name="ph1", padded_shape=[HID, 512])
                        nc.tensor.matmul(
                            ph1[:], wt[f"w1b{l}"][:], o65[:], start=True, stop=False
                        )
                        nc.tensor.matmul(
                            ph1[:], wt[f"b1r{l}"][:], ones1xG[:], start=False, stop=True
                        )
                        h1s = npo.tile([HID, GRP_SLOTS], F32, tag="h1s", name="h1s")
                        nc.vector.tensor_copy(h1s[:], ph1[:])
                        z1s = npo.tile([HID, GRP_SLOTS], F32, tag="z1s", name="z1s")
                        ln_relu(
                            npo, psln, h1s[:], z1s[:], HID,
                            wt[f"g_{l}"][:], wt[f"bn_{l}"][:],
                        )
                        ph2 = psp.tile([H, GRP_SLOTS], F32, tag="ph2", name="ph2", padded_shape=[H, 512])
                        nc.tensor.matmul(
                            ph2[:], wt[f"w2T{l}"][:], z1s[:], start=True, stop=True
                        )
                        if l == 0:
                            nc.vector.tensor_scalar_add(
                                h_res[:, sl], ph2[:], wt["b2_0"][:]
                            )
                        else:
                            tmp = npo.tile([H, GRP_SLOTS], F32, tag="tmp", name="tmp")
                            nc.vector.tensor_scalar_add(
                                tmp[:], ph2[:], wt[f"b2_{l}"][:]
                            )
                            nc.vector.tensor_tensor(
                                h_res[:, sl], h_res[:, sl], tmp[:], OP.add
                            )
                        if l < L - 1:
                            ln_relu(
                                npo, psln, h_res[:, sl], z_res[:, sl], H,
                                wt[f"lng{l+1}"][:], wt[f"lnb{l+1}"][:],
                            )
                        else:
                            qt = npo.tile([H, GRP_SLOTS], F32, tag="qt", name="qt")
                            ln_relu(
                                npo, psln, h_res[:, sl], qt[:], H,
                                wt["lng0"][:], wt["lnb0"][:],
                            )
                            smask_g = npo.tile(
                                [H, GRP_SLOTS], F32, tag="smask_g", name="smask_g"
                            )
                            nc.sync.dma_start(smask_g[:], din["smask"].ap()[:, sl])
                            nc.vector.tensor_tensor(
                                qt[:], qt[:], smask_g[:], OP.add
                            )
                            nc.vector.reduce_max(
                                winmax[:, g * GRP_WIN : (g + 1) * GRP_WIN],
                                qt[:].rearrange("c (w s) -> c w s", s=W_SLOTS),
                                AX.X,
                                op=OP.max,
                            )

                if STOP_AFTER == f"layer{l}":
                    stop_now[0] = True
                    break

            # ============ readout ============
            if stop_now[0]:
                dummy_out()
            with (
                tc.tile_pool(name="ro", bufs=2) as rop,
                tc.tile_pool(name="rops", bufs=1, space="PSUM") as rops,
            ):
              if not stop_now[0]:
                  gm = rop.tile([H, N_GRAPHS, w_prog], F32, tag="gm", name="gm")
                  nc.sync.dma_start(gm[:], din["gmask"].ap())
                  pooled = rop.tile([H, N_GRAPHS], F32, tag="pooled", name="pooled")
                  for gph in range(N_GRAPHS):
                      tmpm = rop.tile([H, w_prog], F32, tag="tmpm", name="tmpm")
                      nc.vector.tensor_tensor(
                          tmpm[:], winmax[:], gm[:, gph, :], OP.add
                      )
                      nc.vector.reduce_max(
                          pooled[:, gph : gph + 1], tmpm[:], AX.X, op=OP.max
                      )
                  nc.sync.dma_start(pool_in.ap(), pooled[:])
                  nc.gpsimd.collective_compute(
                      "AllReduce",
                      OP.max,
                      replica_groups=rg,
                      ins=[pool_in.ap()],
                      outs=[pool_out.ap()],
                  )
                  pool_sb = rop.tile([H, N_GRAPHS], F32, tag="pool_sb", name="pool_sb")
                  nc.sync.dma_start(pool_sb[:], pool_out.ap())
                  pr = rops.tile([N_GRAPHS, 1], F32, tag="pr", name="pr", padded_shape=[N_GRAPHS, 512])
                  nc.tensor.matmul(
                      pr[:], pool_sb[:], wt["ro_w"][:], start=True, stop=True
                  )
                  res_sb = rop.tile([N_GRAPHS, 1], F32, tag="res_sb", name="res_sb")
                  nc.scalar.activation(res_sb[:], pr[:], AF.Sigmoid, bias=wt["ro_b"][:])
                  nc.sync.dma_start(out_d.ap(), res_sb[:])

    nc.compile()
    return nc


def kernel(**inputs):
    edge_index = np.asarray(inputs["edge_index"])
    batch = np.asarray(inputs["batch"])
    n_nodes = np.asarray(inputs["x"]).shape[0]
    gp = build_plan(edge_index, batch, n_nodes)
    core_maps = build_core_inputs(gp, inputs)
    nc = build_nc(gp, np.asarray(inputs["conv_t"], np.float32))
    res = run_bass_kernel_spmd(nc, core_maps, core_ids=list(range(NC)))
    return np.asarray(res.results[0]["out"], np.float32)

